# revision 11
# baseline (speedup 1.0000x reference)
"""Trainium2 Bass kernel for a BasicTransformerBlock (self-attn + cross-attn + GEGLU FFN).

Sharding: pure data-parallel over (batch, query-rows). 8 cores = 2 batches x 4
query-slices of 1024 rows. Only the self-attention K/V path needs all 4096
tokens of a batch element, and K/V are recomputed per core from the (shared)
input x, so there are no collectives at all.

On-device dataflow is kept in "transposed" (feature-on-partition) layout
throughout, which makes every bias/scale a per-partition op and makes the
attention matmuls natural:
  xnT [c,t]  -> qT/kT [d,t] (per head-pair tiles) , v natural [t,d]
  S^T [k,q] = kT^T@qT ; P = exp(S^T/8) ; o65 = [v|1]^T @ P  (denominator rides
  as output row 64) ; normalize with broadcast reciprocal; project, residual,
  LN (stats via gpsimd partition_all_reduce), GEGLU, out-proj.
"""

import numpy as np
import ml_dtypes
from contextlib import ExitStack

import concourse.bass as bass
import concourse.bass_isa as bass_isa
import concourse.tile as tile
from concourse import bacc, mybir
from concourse.bass_utils import run_bass_kernel_spmd

AF = mybir.ActivationFunctionType
BF16 = mybir.dt.bfloat16
F32 = mybir.dt.float32

DIM = 512
H = 8
D = 64
B = 2
S = 4096
TCTX = 77
NCORES = 8
Q = 1024          # query rows per core
P = 128
CC = DIM // P     # contraction chunks of 128
EPS = 1e-5
SCALE = D ** -0.5

_CACHE = {}


def _bcast_dram_ap(ap, nparts):
    """DMA source AP that broadcasts a DRAM row across nparts partitions."""
    return bass.AP(tensor=ap.tensor, offset=ap.offset, ap=[[0, nparts]] + list(ap.ap))


def _body(ctx, tc, a):
    nc = tc.nc
    persist = ctx.enter_context(tc.tile_pool(name="persist", bufs=1))

    def open_pool(name, side="left"):
        cm = tc.tile_pool(name=name, bufs=1, side=side)
        pool = cm.__enter__()
        return cm, pool

    cm_x, pool_x = open_pool("pool_x")          # m_b, r_b, XT : dies after QKV
    eps_t = persist.tile([P, 1], F32, tag="eps")
    nc.vector.memset(eps_t[:], EPS)

    # ---------------- Phase A: LN1 stats (bn_stats over permuted x-natural) ---
    m_b = pool_x.tile([P, S], BF16, tag="m_b")
    r_b = pool_x.tile([P, S], BF16, tag="r_b")
    with tc.tile_pool(name="lnA", bufs=4) as lp:
        mv = lp.tile([P, 32, 2], F32, tag="mv", bufs=1)
        for tb in range(32):
            xt = lp.tile([P, DIM], BF16, tag="xnt")
            nc.sync.dma_start(out=xt, in_=a["xnat"][tb * P:(tb + 1) * P, :])
            st = lp.tile([P, 6], F32, tag="st6")
            nc.vector.bn_stats(out=st, in_=xt)
            nc.vector.bn_aggr(out=mv[:, tb, :], in_=st)
        lnv = lp.tile([P, 32], F32, tag="lnv", bufs=1)
        nc.scalar.activation(out=lnv, in_=mv[:, :, 1], func=AF.Ln, bias=eps_t[:])
        rst = lp.tile([P, 32], BF16, tag="rst", bufs=1)
        nc.scalar.activation(out=rst, in_=lnv, func=AF.Exp, scale=-0.5)
        mbf = lp.tile([P, 32], BF16, tag="mbf", bufs=1)
        nc.vector.tensor_copy(out=mbf, in_=mv[:, :, 0])
        # [128,32] (p-major == t-order thanks to host permute) -> [1,4096] rows
        m_row = lp.tile([1, S], BF16, tag="m_row", bufs=1)
        r_row = lp.tile([1, S], BF16, tag="r_row", bufs=1)
        nc.sync.dma_start(out=m_row, in_=mbf[:])
        nc.sync.dma_start(out=r_row, in_=rst[:])
        nc.gpsimd.partition_broadcast(out_ap=m_b[:], in_ap=m_row[:], channels=P)
        nc.gpsimd.partition_broadcast(out_ap=r_b[:], in_ap=r_row[:], channels=P)

    # ---------------- Phase B: load xT, normalize in place -> xnT -------------
    XT = []
    for c in range(CC):
        t = pool_x.tile([P, S], BF16, tag=f"XT{c}")
        nc.sync.dma_start(out=t, in_=a["xT"][c * P:(c + 1) * P, :])
        XT.append(t)
    for c in range(CC):
        nc.vector.tensor_sub(out=XT[c][:], in0=XT[c][:], in1=m_b[:])
        nc.vector.tensor_mul(out=XT[c][:], in0=XT[c][:], in1=r_b[:])

    # ---------------- Phase C: QKV projections --------------------------------
    cm_at, pool_at = open_pool("pool_at", side="right")  # KT, QT, VP, O1T
    KTP = [pool_at.tile([P, S], BF16, tag=f"KT{p}", name=f"KT{p}") for p in range(4)]
    QTP = [pool_at.tile([P, Q], BF16, tag=f"QT{p}", name=f"QT{p}") for p in range(4)]
    VP = pool_at.tile([P, 32, H, D + 1], BF16, tag="VP")
    nc.vector.memset(VP[:, :, :, D:D + 1], 1.0)

    def load_w(pool, name, rows=DIM, cols=DIM):
        ts = []
        for c in range(rows // P):
            t = pool.tile([P, cols], BF16, tag=f"{name}{c}")
            nc.sync.dma_start(out=t, in_=a[name][c * P:(c + 1) * P, :])
            ts.append(t)
        return ts

    with tc.tile_pool(name="w1", bufs=1) as wp, \
         tc.tile_pool(name="qkvps", bufs=4, space="PSUM") as pp:
        WQ = load_w(wp, "wq1")
        WK = load_w(wp, "wk1")
        WV = load_w(wp, "wv1")
        for p4 in range(4):
            for qb in range(2):
                ps = pp.tile([P, 512], F32, tag="ps")
                for c in range(CC):
                    nc.tensor.matmul(ps[:], lhsT=WQ[c][:, p4 * P:(p4 + 1) * P],
                                     rhs=XT[c][:, qb * 512:(qb + 1) * 512],
                                     start=(c == 0), stop=(c == CC - 1))
                nc.vector.tensor_copy(out=QTP[p4][:, qb * 512:(qb + 1) * 512], in_=ps[:])
            for kb8 in range(8):
                ps = pp.tile([P, 512], F32, tag="ps")
                for c in range(CC):
                    nc.tensor.matmul(ps[:], lhsT=WK[c][:, p4 * P:(p4 + 1) * P],
                                     rhs=XT[c][:, kb8 * 512:(kb8 + 1) * 512],
                                     start=(c == 0), stop=(c == CC - 1))
                if kb8 % 2 == 0:
                    nc.vector.tensor_copy(out=KTP[p4][:, kb8 * 512:(kb8 + 1) * 512], in_=ps[:])
                else:
                    nc.scalar.copy(out=KTP[p4][:, kb8 * 512:(kb8 + 1) * 512], in_=ps[:])
        for tb in range(32):
            ps = pp.tile([P, 512], F32, tag="ps")
            for c in range(CC):
                nc.tensor.matmul(ps[:], lhsT=XT[c][:, tb * P:(tb + 1) * P], rhs=WV[c][:],
                                 start=(c == 0), stop=(c == CC - 1))
            nc.vector.tensor_copy(out=VP[:, tb, :, 0:D],
                                  in_=ps[:].rearrange("p (h d) -> p h d", h=H))

    # ---------------- Phase D: self-attention ---------------------------------
    cm_x.__exit__(None, None, None)   # xnT / m_b / r_b no longer needed
    O1T = pool_at.tile([P, CC, Q], BF16, tag="O1T")
    with tc.tile_pool(name="spool", bufs=3, space="PSUM") as spool, \
         tc.tile_pool(name="opool", bufs=4, space="PSUM") as opool, \
         tc.tile_pool(name="ppool", bufs=4) as ppool, \
         tc.tile_pool(name="npool", bufs=4) as npool:
        for qb in range(2):
            for p4 in range(4):
                oo = []
                for hh in range(2):
                    o65 = opool.tile([D + 1, 512], F32, tag="o65", name=f"o65_{qb}_{p4}_{hh}")
                    oo.append(o65)
                for kb in range(32):
                    for hh in range(2):
                        h = 2 * p4 + hh
                        sA = spool.tile([P, 512], F32, tag="S")
                        nc.tensor.matmul(
                            sA[:],
                            lhsT=KTP[p4][hh * D:(hh + 1) * D, kb * P:(kb + 1) * P],
                            rhs=QTP[p4][hh * D:(hh + 1) * D, qb * 512:(qb + 1) * 512],
                            start=True, stop=True)
                        pA = ppool.tile([P, 512], BF16, tag="P")
                        nc.scalar.activation(out=pA[:], in_=sA[:], func=AF.Exp, scale=SCALE)
                        nc.tensor.matmul(oo[hh][:], lhsT=VP[:, kb, h, :], rhs=pA[:],
                                         start=(kb == 0), stop=(kb == 31))
                for hh in range(2):
                    den = npool.tile([1, 512], F32, tag="den")
                    nc.vector.tensor_copy(out=den[:], in_=oo[hh][D:D + 1, :])
                    dbc = npool.tile([D, 512], F32, tag="dbc")
                    nc.gpsimd.partition_broadcast(out_ap=dbc[:], in_ap=den[:], channels=D)
                    rc = npool.tile([D, 512], F32, tag="rc")
                    nc.vector.reciprocal(out=rc[:], in_=dbc[:])
                    nc.vector.tensor_mul(
                        out=O1T[hh * D:(hh + 1) * D, p4, qb * 512:(qb + 1) * 512],
                        in0=oo[hh][0:D, :], in1=rc[:])

    # ---------------- Phase E: out-proj 1 + residual -> h1T (f32) -------------
    cm_h1, pool_h1 = open_pool("pool_h1")       # XRES + running residual HT (lives to end)
    XRES = []
    for e in range(CC):
        t = pool_h1.tile([P, Q], F32, tag=f"XRES{e}")
        nc.sync.dma_start(out=t, in_=a["xresT"][e * P:(e + 1) * P, :])
        XRES.append(t)

    H1T = pool_h1.tile([P, CC, Q], F32, tag="H1T")
    with tc.tile_pool(name="wo1p", bufs=1) as wp, \
         tc.tile_pool(name="prps", bufs=3, space="PSUM") as pp:
        WO1 = load_w(wp, "wo1")
        for qb in range(2):
            for e in range(CC):
                ps = pp.tile([P, 512], F32, tag="ps")
                for c in range(CC):
                    nc.tensor.matmul(ps[:], lhsT=WO1[c][:, e * P:(e + 1) * P],
                                     rhs=O1T[:, c, qb * 512:(qb + 1) * 512],
                                     start=(c == 0), stop=(c == CC - 1))
                nc.vector.tensor_add(out=H1T[:, e, qb * 512:(qb + 1) * 512],
                                     in0=ps[:], in1=XRES[e][:, qb * 512:(qb + 1) * 512])

    # ---------------- layer norm in transposed layout (stats over partitions) -
    def layer_norm_T(HT, OUT_BF, lp):
        red1 = lp.tile([P, Q], F32, tag="red1")
        red2 = lp.tile([P, Q], F32, tag="red2")
        for e in range(CC):
            r1 = lp.tile([P, Q], F32, tag="r1")
            nc.gpsimd.partition_all_reduce(out_ap=r1[:], in_ap=HT[:, e, :],
                                           channels=P, reduce_op=bass_isa.ReduceOp.add)
            if e == 0:
                nc.vector.tensor_copy(out=red1[:], in_=r1[:])
            else:
                nc.vector.tensor_add(out=red1[:], in0=red1[:], in1=r1[:])
            sq = lp.tile([P, Q], F32, tag="sq")
            nc.vector.tensor_mul(out=sq[:], in0=HT[:, e, :], in1=HT[:, e, :])
            r2 = lp.tile([P, Q], F32, tag="r2")
            nc.gpsimd.partition_all_reduce(out_ap=r2[:], in_ap=sq[:],
                                           channels=P, reduce_op=bass_isa.ReduceOp.add)
            if e == 0:
                nc.vector.tensor_copy(out=red2[:], in_=r2[:])
            else:
                nc.vector.tensor_add(out=red2[:], in0=red2[:], in1=r2[:])
        mm = lp.tile([P, Q], F32, tag="mm")
        nc.vector.tensor_scalar(out=mm[:], in0=red1[:], scalar1=1.0 / DIM, scalar2=None,
                                op0=mybir.AluOpType.mult)
        v1 = lp.tile([P, Q], F32, tag="v1")
        nc.vector.tensor_scalar(out=v1[:], in0=red2[:], scalar1=1.0 / DIM, scalar2=None,
                                op0=mybir.AluOpType.mult)
        m2 = lp.tile([P, Q], F32, tag="m2")
        nc.vector.tensor_mul(out=m2[:], in0=mm[:], in1=mm[:])
        var = lp.tile([P, Q], F32, tag="var")
        nc.vector.tensor_sub(out=var[:], in0=v1[:], in1=m2[:])
        lnv = lp.tile([P, Q], F32, tag="lnv2")
        nc.scalar.activation(out=lnv[:], in_=var[:], func=AF.Ln, bias=eps_t[:])
        rr = lp.tile([P, Q], F32, tag="rr")
        nc.scalar.activation(out=rr[:], in_=lnv[:], func=AF.Exp, scale=-0.5)
        for e in range(CC):
            tmp = lp.tile([P, Q], F32, tag="tmp")
            nc.vector.tensor_sub(out=tmp[:], in0=HT[:, e, :], in1=mm[:])
            nc.vector.tensor_mul(out=OUT_BF[:, e, :], in0=tmp[:], in1=rr[:])

    cm_at.__exit__(None, None, None)  # attention operands done
    cm_mid, pool_mid = open_pool("pool_mid", side="right")  # H1NT, O2T : dies after proj2
    H1NT = pool_mid.tile([P, CC, Q], BF16, tag="H1NT")
    with tc.tile_pool(name="ln2", bufs=1) as lp:
        layer_norm_T(H1T, H1NT, lp)

    # ---------------- Phase F: cross-attention --------------------------------
    O2T = pool_mid.tile([P, CC, Q], BF16, tag="O2T")
    with tc.tile_pool(name="w2", bufs=1) as wp, \
         tc.tile_pool(name="c2ps", bufs=2, space="PSUM") as pp, \
         tc.tile_pool(name="c2sb", bufs=4) as sb:
        WQ2 = load_w(wp, "wq2")
        WK2 = load_w(wp, "wk2")
        WV2 = load_w(wp, "wv2")
        CTX = []
        for c in range(CC):
            t = wp.tile([P, TCTX], BF16, tag=f"CTX{c}")
            nc.sync.dma_start(out=t, in_=a["ctxT"][c * P:(c + 1) * P, :])
            CTX.append(t)

        Q2TP = [wp.tile([P, Q], BF16, tag=f"Q2T{p}", name=f"Q2T{p}") for p in range(4)]
        K2TP = [wp.tile([P, TCTX], BF16, tag=f"K2T{p}", name=f"K2T{p}") for p in range(4)]
        VP2 = wp.tile([TCTX, H, D + 1], BF16, tag="VP2")
        nc.vector.memset(VP2[:, :, D:D + 1], 1.0)

        for p4 in range(4):
            for qb in range(2):
                ps = pp.tile([P, 512], F32, tag="ps2")
                for c in range(CC):
                    nc.tensor.matmul(ps[:], lhsT=WQ2[c][:, p4 * P:(p4 + 1) * P],
                                     rhs=H1NT[:, c, qb * 512:(qb + 1) * 512],
                                     start=(c == 0), stop=(c == CC - 1))
                nc.vector.tensor_copy(out=Q2TP[p4][:, qb * 512:(qb + 1) * 512], in_=ps[:])
            psk = pp.tile([P, TCTX], F32, tag="psk", bufs=1)
            for c in range(CC):
                nc.tensor.matmul(psk[:], lhsT=WK2[c][:, p4 * P:(p4 + 1) * P], rhs=CTX[c][:],
                                 start=(c == 0), stop=(c == CC - 1))
            nc.vector.tensor_copy(out=K2TP[p4][:], in_=psk[:])
        psv = pp.tile([TCTX, 512], F32, tag="psv", bufs=1)
        for c in range(CC):
            nc.tensor.matmul(psv[:], lhsT=CTX[c][:], rhs=WV2[c][:],
                             start=(c == 0), stop=(c == CC - 1))
        nc.vector.tensor_copy(out=VP2[:, :, 0:D],
                              in_=psv[:].rearrange("p (h d) -> p h d", h=H))

        for qb in range(2):
            for h in range(H):
                p4, hh = h // 2, h % 2
                s2 = pp.tile([TCTX, 512], F32, tag="s2")
                nc.tensor.matmul(
                    s2[:],
                    lhsT=K2TP[p4][hh * D:(hh + 1) * D, :],
                    rhs=Q2TP[p4][hh * D:(hh + 1) * D, qb * 512:(qb + 1) * 512],
                    start=True, stop=True)
                p2 = sb.tile([TCTX, 512], BF16, tag="p2")
                nc.scalar.activation(out=p2[:], in_=s2[:], func=AF.Exp, scale=SCALE)
                o65 = pp.tile([D + 1, 512], F32, tag="o65x")
                nc.tensor.matmul(o65[:], lhsT=VP2[:, h, :], rhs=p2[:], start=True, stop=True)
                den = sb.tile([1, 512], F32, tag="den2")
                nc.vector.tensor_copy(out=den[:], in_=o65[D:D + 1, :])
                dbc = sb.tile([D, 512], F32, tag="dbc2")
                nc.gpsimd.partition_broadcast(out_ap=dbc[:], in_ap=den[:], channels=D)
                rc = sb.tile([D, 512], F32, tag="rc2")
                nc.vector.reciprocal(out=rc[:], in_=dbc[:])
                nc.vector.tensor_mul(
                    out=O2T[hh * D:(hh + 1) * D, p4, qb * 512:(qb + 1) * 512],
                    in0=o65[0:D, :], in1=rc[:])

    with tc.tile_pool(name="wo2p", bufs=1) as wp, \
         tc.tile_pool(name="pr2ps", bufs=3, space="PSUM") as pp:
        WO2 = load_w(wp, "wo2")
        for qb in range(2):
            for e in range(CC):
                ps = pp.tile([P, 512], F32, tag="ps")
                for c in range(CC):
                    nc.tensor.matmul(ps[:], lhsT=WO2[c][:, e * P:(e + 1) * P],
                                     rhs=O2T[:, c, qb * 512:(qb + 1) * 512],
                                     start=(c == 0), stop=(c == CC - 1))
                nc.vector.tensor_add(out=H1T[:, e, qb * 512:(qb + 1) * 512],
                                     in0=ps[:], in1=H1T[:, e, qb * 512:(qb + 1) * 512])
    H2T = H1T  # h2 written in place; H1T now holds the post-cross-attn residual

    cm_mid.__exit__(None, None, None)
    cm_ffn, pool_ffn = open_pool("pool_ffn", side="right")  # H2NT, FF : to the end
    H2NT = pool_ffn.tile([P, CC, Q], BF16, tag="H2NT")
    with tc.tile_pool(name="ln3", bufs=1) as lp:
        layer_norm_T(H2T, H2NT, lp)

    # ---------------- Phase G: GEGLU FFN + out proj + residual ----------------
    FB = 16  # 2048/128 blocks in each geglu half
    FF = pool_ffn.tile([P, FB, Q], BF16, tag="FF")
    with tc.tile_pool(name="gwp", bufs=1) as wp, \
         tc.tile_pool(name="ffps", bufs=4, space="PSUM") as pp, \
         tc.tile_pool(name="ffsb", bufs=4) as sb:
        GW = load_w(wp, "gw", cols=8 * DIM)
        for qb in range(2):
            for fb in range(FB):
                psy = pp.tile([P, 512], F32, tag="psy")
                psg = pp.tile([P, 512], F32, tag="psg")
                for c in range(CC):
                    nc.tensor.matmul(psy[:], lhsT=GW[c][:, fb * P:(fb + 1) * P],
                                     rhs=H2NT[:, c, qb * 512:(qb + 1) * 512],
                                     start=(c == 0), stop=(c == CC - 1))
                for c in range(CC):
                    nc.tensor.matmul(psg[:], lhsT=GW[c][:, 4 * DIM + fb * P:4 * DIM + (fb + 1) * P],
                                     rhs=H2NT[:, c, qb * 512:(qb + 1) * 512],
                                     start=(c == 0), stop=(c == CC - 1))
                ga = sb.tile([P, 512], BF16, tag="ga")
                nc.scalar.activation(out=ga[:], in_=psg[:], func=AF.Gelu_apprx_tanh)
                nc.vector.tensor_mul(out=FF[:, fb, qb * 512:(qb + 1) * 512],
                                     in0=psy[:], in1=ga[:])

    with tc.tile_pool(name="owp", bufs=1) as wp, \
         tc.tile_pool(name="outps", bufs=3, space="PSUM") as pp, \
         tc.tile_pool(name="outsb", bufs=3) as sb:
        OW = load_w(wp, "ow", rows=4 * DIM)
        for qb in range(2):
            for e in range(CC):
                ps = pp.tile([P, 512], F32, tag="ps")
                for f in range(FB):
                    nc.tensor.matmul(ps[:], lhsT=OW[f][:, e * P:(e + 1) * P],
                                     rhs=FF[:, f, qb * 512:(qb + 1) * 512],
                                     start=(f == 0), stop=(f == FB - 1))
                fin = sb.tile([P, 512], F32, tag="fin")
                nc.vector.tensor_add(out=fin[:], in0=ps[:],
                                     in1=H2T[:, e, qb * 512:(qb + 1) * 512])
                nc.sync.dma_start(out=a["outT"][e * P:(e + 1) * P, qb * 512:(qb + 1) * 512],
                                  in_=fin[:])

    cm_ffn.__exit__(None, None, None)
    cm_h1.__exit__(None, None, None)


def build_program():
    nc = bacc.Bacc("TRN2", target_bir_lowering=False, debug=False)
    a = {}

    def din(name, shape, dt):
        a[name] = nc.dram_tensor(name, list(shape), dt, kind="ExternalInput").ap()

    din("xT", [DIM, S], BF16)
    din("xnat", [S, DIM], BF16)
    din("xresT", [DIM, Q], F32)
    din("ctxT", [DIM, TCTX], BF16)
    for w in ["wq1", "wk1", "wv1", "wo1", "wq2", "wk2", "wv2", "wo2"]:
        din(w, [DIM, DIM], BF16)
    din("gw", [DIM, 8 * DIM], BF16)
    din("ow", [4 * DIM, DIM], BF16)
    a["outT"] = nc.dram_tensor("outT", [DIM, Q], F32, kind="ExternalOutput").ap()

    with tile.TileContext(nc) as tc:
        with ExitStack() as ctx:
            _body(ctx, tc, a)
    nc.compile()
    return nc


def host_prepare(inputs):
    """Fold LN affine params into weights, cast, slice/permute per core."""
    f = lambda t: np.asarray(t, dtype=np.float32)
    x = f(inputs["x"])
    context = f(inputs["context"])
    g1 = f(inputs["ln1_g"])[:, None]
    g2 = f(inputs["ln2_g"])[:, None]
    g3 = f(inputs["ln3_g"])[:, None]
    for nm in ["ln1_b", "ln2_b", "ln3_b", "bo1", "bo2", "geglu_b", "out_b"]:
        assert not np.any(f(inputs[nm])), f"nonzero bias {nm} not supported"

    bf = ml_dtypes.bfloat16
    weights = {
        "wq1": (g1 * f(inputs["wq1"])).astype(bf),
        "wk1": (g1 * f(inputs["wk1"])).astype(bf),
        "wv1": (g1 * f(inputs["wv1"])).astype(bf),
        "wo1": f(inputs["wo1"]).astype(bf),
        "wq2": (g2 * f(inputs["wq2"])).astype(bf),
        "wk2": f(inputs["wk2"]).astype(bf),
        "wv2": f(inputs["wv2"]).astype(bf),
        "wo2": f(inputs["wo2"]).astype(bf),
        "gw": (g3 * f(inputs["geglu_w"])).astype(bf),
        "ow": f(inputs["out_w"]).astype(bf),
    }

    in_maps = []
    for core in range(NCORES):
        b = core // 4
        q0 = (core % 4) * Q
        perm = np.concatenate([np.arange(q0, q0 + Q), np.delete(np.arange(S), np.s_[q0:q0 + Q])])
        xc = x[b][perm]                       # [S, DIM], own queries first
        m = dict(weights)
        m["xT"] = np.ascontiguousarray(xc.T).astype(bf)
        # bn_stats tile permutation: row tb*128+p holds token p*32+tb
        m["xnat"] = np.ascontiguousarray(
            xc.reshape(P, 32, DIM).transpose(1, 0, 2).reshape(S, DIM)).astype(bf)
        m["xresT"] = np.ascontiguousarray(x[b, q0:q0 + Q].T)
        m["ctxT"] = np.ascontiguousarray(context[b].T).astype(bf)
        in_maps.append(m)
    return in_maps


def kernel(**inputs):
    if "nc" not in _CACHE:
        _CACHE["nc"] = build_program()
    nc = _CACHE["nc"]
    in_maps = host_prepare(inputs)
    res = run_bass_kernel_spmd(nc, in_maps, list(range(NCORES)))
    out = np.zeros((B, S, DIM), dtype=np.float32)
    for core in range(NCORES):
        b = core // 4
        q0 = (core % 4) * Q
        out[b, q0:q0 + Q, :] = res.results[core]["outT"].T
    return out


# revision 12
# speedup vs baseline: 20.6752x; 20.6752x over previous
"""Trainium2 Bass kernel for a BasicTransformerBlock (self-attn + cross-attn + GEGLU FFN).

Sharding: pure data-parallel over (batch, query-rows). 8 cores = 2 batches x 4
query-slices of 1024 rows. Only the self-attention K/V path needs all 4096
tokens of a batch element, and K/V are recomputed per core from the (shared)
input x, so there are no collectives at all.

On-device dataflow is kept in "transposed" (feature-on-partition) layout
throughout, which makes every bias/scale a per-partition op and makes the
attention matmuls natural:
  xnT [c,t]  -> qT/kT [d,t] (per head-pair tiles) , v natural [t,d]
  S^T [k,q] = kT^T@qT ; P = exp(S^T/8) ; o65 = [v|1]^T @ P  (denominator rides
  as output row 64) ; normalize with broadcast reciprocal; project, residual,
  LN (stats via gpsimd partition_all_reduce), GEGLU, out-proj.
"""

import numpy as np
import ml_dtypes
from contextlib import ExitStack

import concourse.bass as bass
import concourse.bass_isa as bass_isa
import concourse.tile as tile
from concourse import bacc, mybir
from concourse.bass_utils import run_bass_kernel_spmd

AF = mybir.ActivationFunctionType
BF16 = mybir.dt.bfloat16
F32 = mybir.dt.float32

DIM = 512
H = 8
D = 64
B = 2
S = 4096
TCTX = 77
NCORES = 8
Q = 1024          # query rows per core
P = 128
CC = DIM // P     # contraction chunks of 128
EPS = 1e-5
SCALE = D ** -0.5

_CACHE = {}


def _bcast_dram_ap(ap, nparts):
    """DMA source AP that broadcasts a DRAM row across nparts partitions."""
    return bass.AP(tensor=ap.tensor, offset=ap.offset, ap=[[0, nparts]] + list(ap.ap))


def _body(ctx, tc, a):
    nc = tc.nc
    persist = ctx.enter_context(tc.tile_pool(name="persist", bufs=1))

    def open_pool(name, side="left"):
        cm = tc.tile_pool(name=name, bufs=1, side=side)
        pool = cm.__enter__()
        return cm, pool

    cm_x, pool_x = open_pool("pool_x")          # m_b, r_b, XT : dies after QKV
    eps_t = persist.tile([P, 1], F32, tag="eps")
    nc.vector.memset(eps_t[:], EPS)

    # ---------------- Phase A: LN1 stats (bn_stats over permuted x-natural) ---
    m_b = pool_x.tile([P, S], BF16, tag="m_b")
    r_b = pool_x.tile([P, S], BF16, tag="r_b")
    with tc.tile_pool(name="lnA", bufs=4) as lp:
        mv = lp.tile([P, 32, 2], F32, tag="mv", bufs=1)
        for tb in range(32):
            xt = lp.tile([P, DIM], BF16, tag="xnt")
            nc.sync.dma_start(out=xt, in_=a["xnat"][tb * P:(tb + 1) * P, :])
            st = lp.tile([P, 6], F32, tag="st6")
            nc.vector.bn_stats(out=st, in_=xt)
            nc.vector.bn_aggr(out=mv[:, tb, :], in_=st)
        lnv = lp.tile([P, 32], F32, tag="lnv", bufs=1)
        nc.scalar.activation(out=lnv, in_=mv[:, :, 1], func=AF.Ln, bias=eps_t[:])
        rst = lp.tile([P, 32], BF16, tag="rst", bufs=1)
        nc.scalar.activation(out=rst, in_=lnv, func=AF.Exp, scale=-0.5)
        mbf = lp.tile([P, 32], BF16, tag="mbf", bufs=1)
        nc.vector.tensor_copy(out=mbf, in_=mv[:, :, 0])
        # [128,32] (p-major == t-order thanks to host permute) -> [1,4096] rows
        m_row = lp.tile([1, S], BF16, tag="m_row", bufs=1)
        r_row = lp.tile([1, S], BF16, tag="r_row", bufs=1)
        nc.sync.dma_start(out=m_row, in_=mbf[:])
        nc.sync.dma_start(out=r_row, in_=rst[:])
        nc.gpsimd.partition_broadcast(out_ap=m_b[:], in_ap=m_row[:], channels=P)
        nc.gpsimd.partition_broadcast(out_ap=r_b[:], in_ap=r_row[:], channels=P)

    # ---------------- Phase B: load xT, normalize in place -> xnT -------------
    XT = []
    for c in range(CC):
        t = pool_x.tile([P, S], BF16, tag=f"XT{c}")
        nc.sync.dma_start(out=t, in_=a["xT"][c * P:(c + 1) * P, :])
        XT.append(t)
    for c in range(CC):
        nc.vector.tensor_sub(out=XT[c][:], in0=XT[c][:], in1=m_b[:])
        nc.vector.tensor_mul(out=XT[c][:], in0=XT[c][:], in1=r_b[:])

    # ---------------- Phase C: QKV projections --------------------------------
    cm_at, pool_at = open_pool("pool_at", side="right")  # KT, QT, VP, O1T
    KTP = [pool_at.tile([P, S], BF16, tag=f"KT{p}", name=f"KT{p}") for p in range(4)]
    QTP = [pool_at.tile([P, Q], BF16, tag=f"QT{p}", name=f"QT{p}") for p in range(4)]
    VP = pool_at.tile([P, 32, H, D + 1], BF16, tag="VP")
    nc.vector.memset(VP[:, :, :, D:D + 1], 1.0)

    def load_w(pool, name, rows=DIM, cols=DIM):
        ts = []
        for c in range(rows // P):
            t = pool.tile([P, cols], BF16, tag=f"{name}{c}")
            nc.sync.dma_start(out=t, in_=a[name][c * P:(c + 1) * P, :])
            ts.append(t)
        return ts

    with tc.tile_pool(name="w1", bufs=1) as wp, \
         tc.tile_pool(name="qkvps", bufs=4, space="PSUM") as pp:
        WQ = load_w(wp, "wq1")
        WK = load_w(wp, "wk1")
        WV = load_w(wp, "wv1")
        for p4 in range(4):
            for qb in range(2):
                ps = pp.tile([P, 512], F32, tag="ps")
                for c in range(CC):
                    nc.tensor.matmul(ps[:], lhsT=WQ[c][:, p4 * P:(p4 + 1) * P],
                                     rhs=XT[c][:, qb * 512:(qb + 1) * 512],
                                     start=(c == 0), stop=(c == CC - 1))
                nc.vector.tensor_copy(out=QTP[p4][:, qb * 512:(qb + 1) * 512], in_=ps[:])
            for kb8 in range(8):
                ps = pp.tile([P, 512], F32, tag="ps")
                for c in range(CC):
                    nc.tensor.matmul(ps[:], lhsT=WK[c][:, p4 * P:(p4 + 1) * P],
                                     rhs=XT[c][:, kb8 * 512:(kb8 + 1) * 512],
                                     start=(c == 0), stop=(c == CC - 1))
                if kb8 % 2 == 0:
                    nc.vector.tensor_copy(out=KTP[p4][:, kb8 * 512:(kb8 + 1) * 512], in_=ps[:])
                else:
                    nc.scalar.copy(out=KTP[p4][:, kb8 * 512:(kb8 + 1) * 512], in_=ps[:])
        for tb in range(32):
            ps = pp.tile([P, 512], F32, tag="ps")
            for c in range(CC):
                nc.tensor.matmul(ps[:], lhsT=XT[c][:, tb * P:(tb + 1) * P], rhs=WV[c][:],
                                 start=(c == 0), stop=(c == CC - 1))
            nc.vector.tensor_copy(out=VP[:, tb, :, 0:D],
                                  in_=ps[:].rearrange("p (h d) -> p h d", h=H))

    # ---------------- Phase D: self-attention ---------------------------------
    cm_x.__exit__(None, None, None)   # xnT / m_b / r_b no longer needed
    O1T = pool_at.tile([P, CC, Q], BF16, tag="O1T")
    with tc.tile_pool(name="spool", bufs=2, space="PSUM") as spool, \
         tc.tile_pool(name="opool", bufs=3, space="PSUM") as opool, \
         tc.tile_pool(name="ppool", bufs=3) as ppool, \
         tc.tile_pool(name="npool", bufs=4) as npool:
        for qb in range(2):
            for p4 in range(4):
                oo = []
                for hh in range(2):
                    o65 = opool.tile([D + 1, 512], F32, tag="o65", name=f"o65_{qb}_{p4}_{hh}")
                    oo.append(o65)
                for kb in range(32):
                    # both heads' scores into one 2-bank psum region, one exp
                    s2t = spool.tile([P, 2, 512], F32, tag="S")
                    for hh in range(2):
                        nc.tensor.matmul(
                            s2t[:, hh, :],
                            lhsT=KTP[p4][hh * D:(hh + 1) * D, kb * P:(kb + 1) * P],
                            rhs=QTP[p4][hh * D:(hh + 1) * D, qb * 512:(qb + 1) * 512],
                            start=True, stop=True)
                    pA = ppool.tile([P, 2, 512], BF16, tag="P")
                    nc.scalar.activation(out=pA[:], in_=s2t[:], func=AF.Exp, scale=SCALE)
                    for hh in range(2):
                        h = 2 * p4 + hh
                        nc.tensor.matmul(oo[hh][:], lhsT=VP[:, kb, h, :], rhs=pA[:, hh, :],
                                         start=(kb == 0), stop=(kb == 31))
                for hh in range(2):
                    den = npool.tile([1, 512], F32, tag="den")
                    nc.vector.tensor_copy(out=den[:], in_=oo[hh][D:D + 1, :])
                    dbc = npool.tile([D, 512], F32, tag="dbc")
                    nc.gpsimd.partition_broadcast(out_ap=dbc[:], in_ap=den[:], channels=D)
                    rc = npool.tile([D, 512], F32, tag="rc")
                    nc.vector.reciprocal(out=rc[:], in_=dbc[:])
                    nc.vector.tensor_mul(
                        out=O1T[hh * D:(hh + 1) * D, p4, qb * 512:(qb + 1) * 512],
                        in0=oo[hh][0:D, :], in1=rc[:])

    # ---------------- Phase E: out-proj 1 + residual -> h1T (f32) -------------
    cm_h1, pool_h1 = open_pool("pool_h1")       # XRES + running residual HT (lives to end)
    XRES = []
    for e in range(CC):
        t = pool_h1.tile([P, Q], F32, tag=f"XRES{e}")
        nc.sync.dma_start(out=t, in_=a["xresT"][e * P:(e + 1) * P, :])
        XRES.append(t)

    H1T = pool_h1.tile([P, CC, Q], F32, tag="H1T")
    with tc.tile_pool(name="wo1p", bufs=1) as wp, \
         tc.tile_pool(name="prps", bufs=3, space="PSUM") as pp:
        WO1 = load_w(wp, "wo1")
        for qb in range(2):
            for e in range(CC):
                ps = pp.tile([P, 512], F32, tag="ps")
                for c in range(CC):
                    nc.tensor.matmul(ps[:], lhsT=WO1[c][:, e * P:(e + 1) * P],
                                     rhs=O1T[:, c, qb * 512:(qb + 1) * 512],
                                     start=(c == 0), stop=(c == CC - 1))
                nc.vector.tensor_add(out=H1T[:, e, qb * 512:(qb + 1) * 512],
                                     in0=ps[:], in1=XRES[e][:, qb * 512:(qb + 1) * 512])

    # ---------------- layer norm in transposed layout (stats over partitions) -
    def layer_norm_T(HT, OUT_BF, lp):
        red1 = lp.tile([P, Q], F32, tag="red1")
        red2 = lp.tile([P, Q], F32, tag="red2")
        for e in range(CC):
            r1 = lp.tile([P, Q], F32, tag="r1")
            nc.gpsimd.partition_all_reduce(out_ap=r1[:], in_ap=HT[:, e, :],
                                           channels=P, reduce_op=bass_isa.ReduceOp.add)
            if e == 0:
                nc.vector.tensor_copy(out=red1[:], in_=r1[:])
            else:
                nc.vector.tensor_add(out=red1[:], in0=red1[:], in1=r1[:])
            sq = lp.tile([P, Q], F32, tag="sq")
            nc.vector.tensor_mul(out=sq[:], in0=HT[:, e, :], in1=HT[:, e, :])
            r2 = lp.tile([P, Q], F32, tag="r2")
            nc.gpsimd.partition_all_reduce(out_ap=r2[:], in_ap=sq[:],
                                           channels=P, reduce_op=bass_isa.ReduceOp.add)
            if e == 0:
                nc.vector.tensor_copy(out=red2[:], in_=r2[:])
            else:
                nc.vector.tensor_add(out=red2[:], in0=red2[:], in1=r2[:])
        mm = lp.tile([P, Q], F32, tag="mm")
        nc.vector.tensor_scalar(out=mm[:], in0=red1[:], scalar1=1.0 / DIM, scalar2=None,
                                op0=mybir.AluOpType.mult)
        v1 = lp.tile([P, Q], F32, tag="v1")
        nc.vector.tensor_scalar(out=v1[:], in0=red2[:], scalar1=1.0 / DIM, scalar2=None,
                                op0=mybir.AluOpType.mult)
        m2 = lp.tile([P, Q], F32, tag="m2")
        nc.vector.tensor_mul(out=m2[:], in0=mm[:], in1=mm[:])
        var = lp.tile([P, Q], F32, tag="var")
        nc.vector.tensor_sub(out=var[:], in0=v1[:], in1=m2[:])
        lnv = lp.tile([P, Q], F32, tag="lnv2")
        nc.scalar.activation(out=lnv[:], in_=var[:], func=AF.Ln, bias=eps_t[:])
        rr = lp.tile([P, Q], F32, tag="rr")
        nc.scalar.activation(out=rr[:], in_=lnv[:], func=AF.Exp, scale=-0.5)
        for e in range(CC):
            tmp = lp.tile([P, Q], F32, tag="tmp")
            nc.vector.tensor_sub(out=tmp[:], in0=HT[:, e, :], in1=mm[:])
            nc.vector.tensor_mul(out=OUT_BF[:, e, :], in0=tmp[:], in1=rr[:])

    cm_at.__exit__(None, None, None)  # attention operands done
    cm_mid, pool_mid = open_pool("pool_mid", side="right")  # H1NT, O2T : dies after proj2
    H1NT = pool_mid.tile([P, CC, Q], BF16, tag="H1NT")
    with tc.tile_pool(name="ln2", bufs=1) as lp:
        layer_norm_T(H1T, H1NT, lp)

    # ---------------- Phase F: cross-attention --------------------------------
    O2T = pool_mid.tile([P, CC, Q], BF16, tag="O2T")
    with tc.tile_pool(name="w2", bufs=1) as wp, \
         tc.tile_pool(name="c2ps", bufs=2, space="PSUM") as pp, \
         tc.tile_pool(name="c2sb", bufs=4) as sb:
        WQ2 = load_w(wp, "wq2")
        WK2 = load_w(wp, "wk2")
        WV2 = load_w(wp, "wv2")
        CTX = []
        for c in range(CC):
            t = wp.tile([P, TCTX], BF16, tag=f"CTX{c}")
            nc.sync.dma_start(out=t, in_=a["ctxT"][c * P:(c + 1) * P, :])
            CTX.append(t)

        Q2TP = [wp.tile([P, Q], BF16, tag=f"Q2T{p}", name=f"Q2T{p}") for p in range(4)]
        K2TP = [wp.tile([P, TCTX], BF16, tag=f"K2T{p}", name=f"K2T{p}") for p in range(4)]
        VP2 = wp.tile([TCTX, H, D + 1], BF16, tag="VP2")
        nc.vector.memset(VP2[:, :, D:D + 1], 1.0)

        for p4 in range(4):
            for qb in range(2):
                ps = pp.tile([P, 512], F32, tag="ps2")
                for c in range(CC):
                    nc.tensor.matmul(ps[:], lhsT=WQ2[c][:, p4 * P:(p4 + 1) * P],
                                     rhs=H1NT[:, c, qb * 512:(qb + 1) * 512],
                                     start=(c == 0), stop=(c == CC - 1))
                nc.vector.tensor_copy(out=Q2TP[p4][:, qb * 512:(qb + 1) * 512], in_=ps[:])
            psk = pp.tile([P, TCTX], F32, tag="psk", bufs=1)
            for c in range(CC):
                nc.tensor.matmul(psk[:], lhsT=WK2[c][:, p4 * P:(p4 + 1) * P], rhs=CTX[c][:],
                                 start=(c == 0), stop=(c == CC - 1))
            nc.vector.tensor_copy(out=K2TP[p4][:], in_=psk[:])
        psv = pp.tile([TCTX, 512], F32, tag="psv", bufs=1)
        for c in range(CC):
            nc.tensor.matmul(psv[:], lhsT=CTX[c][:], rhs=WV2[c][:],
                             start=(c == 0), stop=(c == CC - 1))
        nc.vector.tensor_copy(out=VP2[:, :, 0:D],
                              in_=psv[:].rearrange("p (h d) -> p h d", h=H))

        for qb in range(2):
            for h in range(H):
                p4, hh = h // 2, h % 2
                s2 = pp.tile([TCTX, 512], F32, tag="s2")
                nc.tensor.matmul(
                    s2[:],
                    lhsT=K2TP[p4][hh * D:(hh + 1) * D, :],
                    rhs=Q2TP[p4][hh * D:(hh + 1) * D, qb * 512:(qb + 1) * 512],
                    start=True, stop=True)
                p2 = sb.tile([TCTX, 512], BF16, tag="p2")
                nc.scalar.activation(out=p2[:], in_=s2[:], func=AF.Exp, scale=SCALE)
                o65 = pp.tile([D + 1, 512], F32, tag="o65x")
                nc.tensor.matmul(o65[:], lhsT=VP2[:, h, :], rhs=p2[:], start=True, stop=True)
                den = sb.tile([1, 512], F32, tag="den2")
                nc.vector.tensor_copy(out=den[:], in_=o65[D:D + 1, :])
                dbc = sb.tile([D, 512], F32, tag="dbc2")
                nc.gpsimd.partition_broadcast(out_ap=dbc[:], in_ap=den[:], channels=D)
                rc = sb.tile([D, 512], F32, tag="rc2")
                nc.vector.reciprocal(out=rc[:], in_=dbc[:])
                nc.vector.tensor_mul(
                    out=O2T[hh * D:(hh + 1) * D, p4, qb * 512:(qb + 1) * 512],
                    in0=o65[0:D, :], in1=rc[:])

    with tc.tile_pool(name="wo2p", bufs=1) as wp, \
         tc.tile_pool(name="pr2ps", bufs=3, space="PSUM") as pp:
        WO2 = load_w(wp, "wo2")
        for qb in range(2):
            for e in range(CC):
                ps = pp.tile([P, 512], F32, tag="ps")
                for c in range(CC):
                    nc.tensor.matmul(ps[:], lhsT=WO2[c][:, e * P:(e + 1) * P],
                                     rhs=O2T[:, c, qb * 512:(qb + 1) * 512],
                                     start=(c == 0), stop=(c == CC - 1))
                nc.vector.tensor_add(out=H1T[:, e, qb * 512:(qb + 1) * 512],
                                     in0=ps[:], in1=H1T[:, e, qb * 512:(qb + 1) * 512])
    H2T = H1T  # h2 written in place; H1T now holds the post-cross-attn residual

    cm_mid.__exit__(None, None, None)
    cm_ffn, pool_ffn = open_pool("pool_ffn", side="right")  # H2NT, FF : to the end
    H2NT = pool_ffn.tile([P, CC, Q], BF16, tag="H2NT")
    with tc.tile_pool(name="ln3", bufs=1) as lp:
        layer_norm_T(H2T, H2NT, lp)

    # ---------------- Phase G: GEGLU FFN + out proj + residual ----------------
    FB = 16  # 2048/128 blocks in each geglu half
    FF = pool_ffn.tile([P, FB, Q], BF16, tag="FF")
    with tc.tile_pool(name="gwp", bufs=1) as wp, \
         tc.tile_pool(name="ffps", bufs=4, space="PSUM") as pp, \
         tc.tile_pool(name="ffsb", bufs=4) as sb:
        GW = load_w(wp, "gw", cols=8 * DIM)
        for qb in range(2):
            for fb in range(FB):
                psy = pp.tile([P, 512], F32, tag="psy")
                psg = pp.tile([P, 512], F32, tag="psg")
                for c in range(CC):
                    nc.tensor.matmul(psy[:], lhsT=GW[c][:, fb * P:(fb + 1) * P],
                                     rhs=H2NT[:, c, qb * 512:(qb + 1) * 512],
                                     start=(c == 0), stop=(c == CC - 1))
                for c in range(CC):
                    nc.tensor.matmul(psg[:], lhsT=GW[c][:, 4 * DIM + fb * P:4 * DIM + (fb + 1) * P],
                                     rhs=H2NT[:, c, qb * 512:(qb + 1) * 512],
                                     start=(c == 0), stop=(c == CC - 1))
                ga = sb.tile([P, 512], BF16, tag="ga")
                nc.scalar.activation(out=ga[:], in_=psg[:], func=AF.Gelu_apprx_tanh)
                nc.vector.tensor_mul(out=FF[:, fb, qb * 512:(qb + 1) * 512],
                                     in0=psy[:], in1=ga[:])

    with tc.tile_pool(name="owp", bufs=1) as wp, \
         tc.tile_pool(name="outps", bufs=3, space="PSUM") as pp, \
         tc.tile_pool(name="outsb", bufs=3) as sb:
        OW = load_w(wp, "ow", rows=4 * DIM)
        for qb in range(2):
            for e in range(CC):
                ps = pp.tile([P, 512], F32, tag="ps")
                for f in range(FB):
                    nc.tensor.matmul(ps[:], lhsT=OW[f][:, e * P:(e + 1) * P],
                                     rhs=FF[:, f, qb * 512:(qb + 1) * 512],
                                     start=(f == 0), stop=(f == FB - 1))
                fin = sb.tile([P, 512], F32, tag="fin")
                nc.vector.tensor_add(out=fin[:], in0=ps[:],
                                     in1=H2T[:, e, qb * 512:(qb + 1) * 512])
                nc.sync.dma_start(out=a["outT"][e * P:(e + 1) * P, qb * 512:(qb + 1) * 512],
                                  in_=fin[:])

    cm_ffn.__exit__(None, None, None)
    cm_h1.__exit__(None, None, None)


def build_program():
    nc = bacc.Bacc("TRN2", target_bir_lowering=False, debug=False)
    a = {}

    def din(name, shape, dt):
        a[name] = nc.dram_tensor(name, list(shape), dt, kind="ExternalInput").ap()

    din("xT", [DIM, S], BF16)
    din("xnat", [S, DIM], BF16)
    din("xresT", [DIM, Q], F32)
    din("ctxT", [DIM, TCTX], BF16)
    for w in ["wq1", "wk1", "wv1", "wo1", "wq2", "wk2", "wv2", "wo2"]:
        din(w, [DIM, DIM], BF16)
    din("gw", [DIM, 8 * DIM], BF16)
    din("ow", [4 * DIM, DIM], BF16)
    a["outT"] = nc.dram_tensor("outT", [DIM, Q], F32, kind="ExternalOutput").ap()

    with tile.TileContext(nc) as tc:
        with ExitStack() as ctx:
            _body(ctx, tc, a)
    nc.compile()
    return nc


def host_prepare(inputs):
    """Fold LN affine params into weights, cast, slice/permute per core."""
    f = lambda t: np.asarray(t, dtype=np.float32)
    x = f(inputs["x"])
    context = f(inputs["context"])
    g1 = f(inputs["ln1_g"])[:, None]
    g2 = f(inputs["ln2_g"])[:, None]
    g3 = f(inputs["ln3_g"])[:, None]
    for nm in ["ln1_b", "ln2_b", "ln3_b", "bo1", "bo2", "geglu_b", "out_b"]:
        assert not np.any(f(inputs[nm])), f"nonzero bias {nm} not supported"

    bf = ml_dtypes.bfloat16
    weights = {
        "wq1": (g1 * f(inputs["wq1"])).astype(bf),
        "wk1": (g1 * f(inputs["wk1"])).astype(bf),
        "wv1": (g1 * f(inputs["wv1"])).astype(bf),
        "wo1": f(inputs["wo1"]).astype(bf),
        "wq2": (g2 * f(inputs["wq2"])).astype(bf),
        "wk2": f(inputs["wk2"]).astype(bf),
        "wv2": f(inputs["wv2"]).astype(bf),
        "wo2": f(inputs["wo2"]).astype(bf),
        "gw": (g3 * f(inputs["geglu_w"])).astype(bf),
        "ow": f(inputs["out_w"]).astype(bf),
    }

    in_maps = []
    for core in range(NCORES):
        b = core // 4
        q0 = (core % 4) * Q
        perm = np.concatenate([np.arange(q0, q0 + Q), np.delete(np.arange(S), np.s_[q0:q0 + Q])])
        xc = x[b][perm]                       # [S, DIM], own queries first
        m = dict(weights)
        m["xT"] = np.ascontiguousarray(xc.T).astype(bf)
        # bn_stats tile permutation: row tb*128+p holds token p*32+tb
        m["xnat"] = np.ascontiguousarray(
            xc.reshape(P, 32, DIM).transpose(1, 0, 2).reshape(S, DIM)).astype(bf)
        m["xresT"] = np.ascontiguousarray(x[b, q0:q0 + Q].T)
        m["ctxT"] = np.ascontiguousarray(context[b].T).astype(bf)
        in_maps.append(m)
    return in_maps


def kernel(**inputs):
    if "nc" not in _CACHE:
        _CACHE["nc"] = build_program()
    nc = _CACHE["nc"]
    in_maps = host_prepare(inputs)
    res = run_bass_kernel_spmd(nc, in_maps, list(range(NCORES)))
    out = np.zeros((B, S, DIM), dtype=np.float32)
    for core in range(NCORES):
        b = core // 4
        q0 = (core % 4) * Q
        out[b, q0:q0 + Q, :] = res.results[core]["outT"].T
    return out


# revision 14
# speedup vs baseline: 25.8139x; 1.2485x over previous
"""Trainium2 Bass kernel for a BasicTransformerBlock (self-attn + cross-attn + GEGLU FFN).

Sharding: pure data-parallel over (batch, query-rows). 8 cores = 2 batches x 4
query-slices of 1024 rows. Only the self-attention K/V path needs all 4096
tokens of a batch element, and K/V are recomputed per core from the (shared)
input x, so there are no collectives at all.

On-device dataflow is kept in "transposed" (feature-on-partition) layout
throughout, which makes every bias/scale a per-partition op and makes the
attention matmuls natural:
  xnT [c,t]  -> qT/kT [d,t] (per head-pair tiles) , v natural [t,d]
  S^T [k,q] = kT^T@qT ; P = exp(S^T/8) ; o65 = [v|1]^T @ P  (denominator rides
  as output row 64) ; normalize with broadcast reciprocal; project, residual,
  LN (stats via gpsimd partition_all_reduce), GEGLU, out-proj.
"""

import numpy as np
import ml_dtypes
from contextlib import ExitStack

import concourse.bass as bass
import concourse.bass_isa as bass_isa
import concourse.tile as tile
from concourse import bacc, mybir
from concourse.bass_utils import run_bass_kernel_spmd

AF = mybir.ActivationFunctionType
BF16 = mybir.dt.bfloat16
F32 = mybir.dt.float32

DIM = 512
H = 8
D = 64
B = 2
S = 4096
TCTX = 77
NCORES = 8
Q = 1024          # query rows per core
P = 128
CC = DIM // P     # contraction chunks of 128
EPS = 1e-5
SCALE = D ** -0.5

_CACHE = {}


def _bcast_dram_ap(ap, nparts):
    """DMA source AP that broadcasts a DRAM row across nparts partitions."""
    return bass.AP(tensor=ap.tensor, offset=ap.offset, ap=[[0, nparts]] + list(ap.ap))


def _body(ctx, tc, a):
    nc = tc.nc
    persist = ctx.enter_context(tc.tile_pool(name="persist", bufs=1))

    def open_pool(name, side="left"):
        cm = tc.tile_pool(name=name, bufs=1, side=side)
        pool = cm.__enter__()
        return cm, pool

    cm_x, pool_x = open_pool("pool_x")          # m_b, r_b, XT : dies after QKV
    eps_t = persist.tile([P, 1], F32, tag="eps")
    nc.vector.memset(eps_t[:], EPS)

    # ---------------- Phase A: LN1 stats (bn_stats over permuted x-natural) ---
    m_b = pool_x.tile([P, S], BF16, tag="m_b")
    r_b = pool_x.tile([P, S], BF16, tag="r_b")
    with tc.tile_pool(name="lnA", bufs=4) as lp:
        mv = lp.tile([P, 32, 2], F32, tag="mv", bufs=1)
        for tb in range(32):
            xt = lp.tile([P, DIM], BF16, tag="xnt")
            nc.sync.dma_start(out=xt, in_=a["xnat"][tb * P:(tb + 1) * P, :])
            st = lp.tile([P, 6], F32, tag="st6")
            nc.vector.bn_stats(out=st, in_=xt)
            nc.vector.bn_aggr(out=mv[:, tb, :], in_=st)
        lnv = lp.tile([P, 32], F32, tag="lnv", bufs=1)
        nc.scalar.activation(out=lnv, in_=mv[:, :, 1], func=AF.Ln, bias=eps_t[:])
        rst = lp.tile([P, 32], BF16, tag="rst", bufs=1)
        nc.scalar.activation(out=rst, in_=lnv, func=AF.Exp, scale=-0.5)
        mbf = lp.tile([P, 32], BF16, tag="mbf", bufs=1)
        nc.vector.tensor_copy(out=mbf, in_=mv[:, :, 0])
        # [128,32] (p-major == t-order thanks to host permute) -> [1,4096] rows
        m_row = lp.tile([1, S], BF16, tag="m_row", bufs=1)
        r_row = lp.tile([1, S], BF16, tag="r_row", bufs=1)
        nc.sync.dma_start(out=m_row, in_=mbf[:])
        nc.sync.dma_start(out=r_row, in_=rst[:])
        nc.gpsimd.partition_broadcast(out_ap=m_b[:], in_ap=m_row[:], channels=P)
        nc.gpsimd.partition_broadcast(out_ap=r_b[:], in_ap=r_row[:], channels=P)

    # ---------------- Phase B: load xT, normalize in place -> xnT -------------
    XT = []
    for c in range(CC):
        t = pool_x.tile([P, S], BF16, tag=f"XT{c}")
        nc.sync.dma_start(out=t, in_=a["xT"][c * P:(c + 1) * P, :])
        XT.append(t)
    for c in range(CC):
        nc.vector.tensor_sub(out=XT[c][:], in0=XT[c][:], in1=m_b[:])
        nc.vector.tensor_mul(out=XT[c][:], in0=XT[c][:], in1=r_b[:])

    # ---------------- Phase C: QKV projections --------------------------------
    cm_at, pool_at = open_pool("pool_at", side="right")  # KT, QT, VP, O1T
    KTP = [pool_at.tile([P, S], BF16, tag=f"KT{p}", name=f"KT{p}") for p in range(4)]
    QTP = [pool_at.tile([P, Q], BF16, tag=f"QT{p}", name=f"QT{p}") for p in range(4)]
    VP = pool_at.tile([P, 32, H, D + 1], BF16, tag="VP")
    nc.vector.memset(VP[:, :, :, D:D + 1], 1.0)

    def load_w(pool, name, rows=DIM, cols=DIM):
        ts = []
        for c in range(rows // P):
            t = pool.tile([P, cols], BF16, tag=f"{name}{c}")
            nc.sync.dma_start(out=t, in_=a[name][c * P:(c + 1) * P, :])
            ts.append(t)
        return ts

    with tc.tile_pool(name="w1", bufs=1) as wp, \
         tc.tile_pool(name="qkvps", bufs=4, space="PSUM") as pp:
        WQ = load_w(wp, "wq1")
        WK = load_w(wp, "wk1")
        WV = load_w(wp, "wv1")
        for p4 in range(4):
            for qb in range(2):
                ps = pp.tile([P, 512], F32, tag="ps")
                for c in range(CC):
                    nc.tensor.matmul(ps[:], lhsT=WQ[c][:, p4 * P:(p4 + 1) * P],
                                     rhs=XT[c][:, qb * 512:(qb + 1) * 512],
                                     start=(c == 0), stop=(c == CC - 1))
                nc.vector.tensor_copy(out=QTP[p4][:, qb * 512:(qb + 1) * 512], in_=ps[:])
            for kb8 in range(8):
                ps = pp.tile([P, 512], F32, tag="ps")
                for c in range(CC):
                    nc.tensor.matmul(ps[:], lhsT=WK[c][:, p4 * P:(p4 + 1) * P],
                                     rhs=XT[c][:, kb8 * 512:(kb8 + 1) * 512],
                                     start=(c == 0), stop=(c == CC - 1))
                if kb8 % 2 == 0:
                    nc.vector.tensor_copy(out=KTP[p4][:, kb8 * 512:(kb8 + 1) * 512], in_=ps[:])
                else:
                    nc.scalar.copy(out=KTP[p4][:, kb8 * 512:(kb8 + 1) * 512], in_=ps[:])
        for tb in range(32):
            ps = pp.tile([P, 512], F32, tag="ps")
            for c in range(CC):
                nc.tensor.matmul(ps[:], lhsT=XT[c][:, tb * P:(tb + 1) * P], rhs=WV[c][:],
                                 start=(c == 0), stop=(c == CC - 1))
            nc.vector.tensor_copy(out=VP[:, tb, :, 0:D],
                                  in_=ps[:].rearrange("p (h d) -> p h d", h=H))

    # ---------------- Phase D: self-attention ---------------------------------
    cm_x.__exit__(None, None, None)   # xnT / m_b / r_b no longer needed
    O1T = pool_at.tile([P, CC, Q], BF16, tag="O1T")
    with tc.tile_pool(name="spool", bufs=2, space="PSUM") as spool, \
         tc.tile_pool(name="opool", bufs=3, space="PSUM") as opool, \
         tc.tile_pool(name="ppool", bufs=3) as ppool, \
         tc.tile_pool(name="npool", bufs=4) as npool:
        for qb in range(2):
            for p4 in range(4):
                oo = []
                for hh in range(2):
                    o65 = opool.tile([D + 1, 512], F32, tag="o65", name=f"o65_{qb}_{p4}_{hh}")
                    oo.append(o65)
                pend = None  # software pipeline: attnV(kb) issues after scores(kb+1)
                for kb in range(32):
                    # both heads' scores into one 2-bank psum region, one exp
                    s2t = spool.tile([P, 2, 512], F32, tag="S")
                    for hh in range(2):
                        nc.tensor.matmul(
                            s2t[:, hh, :],
                            lhsT=KTP[p4][hh * D:(hh + 1) * D, kb * P:(kb + 1) * P],
                            rhs=QTP[p4][hh * D:(hh + 1) * D, qb * 512:(qb + 1) * 512],
                            start=True, stop=True)
                    pA = ppool.tile([P, 2, 512], BF16, tag="P")
                    nc.scalar.activation(out=pA[:], in_=s2t[:], func=AF.Exp, scale=SCALE)
                    if pend is not None:
                        pkb, ppA = pend
                        for hh in range(2):
                            nc.tensor.matmul(oo[hh][:], lhsT=VP[:, pkb, 2 * p4 + hh, :],
                                             rhs=ppA[:, hh, :],
                                             start=(pkb == 0), stop=False)
                    pend = (kb, pA)
                pkb, ppA = pend
                for hh in range(2):
                    nc.tensor.matmul(oo[hh][:], lhsT=VP[:, pkb, 2 * p4 + hh, :],
                                     rhs=ppA[:, hh, :], start=False, stop=True)
                for hh in range(2):
                    den = npool.tile([1, 512], F32, tag="den")
                    nc.vector.tensor_copy(out=den[:], in_=oo[hh][D:D + 1, :])
                    dbc = npool.tile([D, 512], F32, tag="dbc")
                    nc.gpsimd.partition_broadcast(out_ap=dbc[:], in_ap=den[:], channels=D)
                    rc = npool.tile([D, 512], F32, tag="rc")
                    nc.vector.reciprocal_approx_fast(out=rc[:], in_=dbc[:])
                    nc.vector.tensor_mul(
                        out=O1T[hh * D:(hh + 1) * D, p4, qb * 512:(qb + 1) * 512],
                        in0=oo[hh][0:D, :], in1=rc[:])

    # ---------------- Phase E: out-proj 1 + residual -> h1T (f32) -------------
    cm_h1, pool_h1 = open_pool("pool_h1")       # XRES + running residual HT (lives to end)
    XRES = []
    for e in range(CC):
        t = pool_h1.tile([P, Q], F32, tag=f"XRES{e}")
        nc.sync.dma_start(out=t, in_=a["xresT"][e * P:(e + 1) * P, :])
        XRES.append(t)

    H1T = pool_h1.tile([P, CC, Q], F32, tag="H1T")
    with tc.tile_pool(name="wo1p", bufs=1) as wp, \
         tc.tile_pool(name="prps", bufs=3, space="PSUM") as pp:
        WO1 = load_w(wp, "wo1")
        for qb in range(2):
            for e in range(CC):
                ps = pp.tile([P, 512], F32, tag="ps")
                for c in range(CC):
                    nc.tensor.matmul(ps[:], lhsT=WO1[c][:, e * P:(e + 1) * P],
                                     rhs=O1T[:, c, qb * 512:(qb + 1) * 512],
                                     start=(c == 0), stop=(c == CC - 1))
                nc.vector.tensor_add(out=H1T[:, e, qb * 512:(qb + 1) * 512],
                                     in0=ps[:], in1=XRES[e][:, qb * 512:(qb + 1) * 512])

    # ---------------- layer norm in transposed layout (stats over partitions) -
    ones_b = persist.tile([P, 1], BF16, tag="ones_b")
    nc.vector.memset(ones_b[:], 1.0)

    def layer_norm_T(HT, OUT_BF, lp):
        # per-token sums over the feature axis via ones-stationary matmuls
        # (partition reduce on PE); stats stay token-on-free so the broadcast
        # is a plain partition_broadcast.
        with tc.tile_pool(name="lnps", bufs=1, space="PSUM") as pp:
            ps1 = pp.tile([1, Q], F32, tag="lnps1")
            ps2 = pp.tile([1, Q], F32, tag="lnps2")
            for c in range(CC):
                hbf = lp.tile([P, Q], BF16, tag="hbf")
                nc.vector.tensor_copy(out=hbf[:], in_=HT[:, c, :])
                sq = lp.tile([P, Q], BF16, tag="sq")
                nc.vector.tensor_mul(out=sq[:], in0=hbf[:], in1=hbf[:])
                for qh in range(2):
                    nc.tensor.matmul(ps1[:, qh * 512:(qh + 1) * 512], lhsT=ones_b[:],
                                     rhs=hbf[:, qh * 512:(qh + 1) * 512],
                                     start=(c == 0), stop=(c == CC - 1))
                    nc.tensor.matmul(ps2[:, qh * 512:(qh + 1) * 512], lhsT=ones_b[:],
                                     rhs=sq[:, qh * 512:(qh + 1) * 512],
                                     start=(c == 0), stop=(c == CC - 1))
            m_row = lp.tile([1, Q], F32, tag="m_row2")
            nc.vector.tensor_scalar(out=m_row[:], in0=ps1[:], scalar1=1.0 / DIM,
                                    scalar2=None, op0=mybir.AluOpType.mult)
            v1 = lp.tile([1, Q], F32, tag="v1r")
            nc.vector.tensor_scalar(out=v1[:], in0=ps2[:], scalar1=1.0 / DIM,
                                    scalar2=None, op0=mybir.AluOpType.mult)
            m2 = lp.tile([1, Q], F32, tag="m2r")
            nc.vector.tensor_mul(out=m2[:], in0=m_row[:], in1=m_row[:])
            var = lp.tile([1, Q], F32, tag="varr")
            nc.vector.tensor_sub(out=var[:], in0=v1[:], in1=m2[:])
            lnv = lp.tile([1, Q], F32, tag="lnvr")
            nc.scalar.activation(out=lnv[:], in_=var[:], func=AF.Ln, bias=eps_t[0:1, :])
            r_row = lp.tile([1, Q], F32, tag="r_row2")
            nc.scalar.activation(out=r_row[:], in_=lnv[:], func=AF.Exp, scale=-0.5)
        mb = lp.tile([P, Q], F32, tag="mb2")
        rb = lp.tile([P, Q], F32, tag="rb2")
        nc.gpsimd.partition_broadcast(out_ap=mb[:], in_ap=m_row[:], channels=P)
        nc.gpsimd.partition_broadcast(out_ap=rb[:], in_ap=r_row[:], channels=P)
        for e in range(CC):
            tmp = lp.tile([P, Q], F32, tag="tmp")
            nc.vector.tensor_sub(out=tmp[:], in0=HT[:, e, :], in1=mb[:])
            nc.vector.tensor_mul(out=OUT_BF[:, e, :], in0=tmp[:], in1=rb[:])

    cm_at.__exit__(None, None, None)  # attention operands done
    cm_mid, pool_mid = open_pool("pool_mid", side="right")  # H1NT, O2T : dies after proj2
    H1NT = pool_mid.tile([P, CC, Q], BF16, tag="H1NT")
    with tc.tile_pool(name="ln2", bufs=1) as lp:
        layer_norm_T(H1T, H1NT, lp)

    # ---------------- Phase F: cross-attention --------------------------------
    O2T = pool_mid.tile([P, CC, Q], BF16, tag="O2T")
    with tc.tile_pool(name="w2", bufs=1) as wp, \
         tc.tile_pool(name="c2ps", bufs=2, space="PSUM") as pp, \
         tc.tile_pool(name="c2sb", bufs=4) as sb:
        WQ2 = load_w(wp, "wq2")
        WK2 = load_w(wp, "wk2")
        WV2 = load_w(wp, "wv2")
        CTX = []
        for c in range(CC):
            t = wp.tile([P, TCTX], BF16, tag=f"CTX{c}")
            nc.sync.dma_start(out=t, in_=a["ctxT"][c * P:(c + 1) * P, :])
            CTX.append(t)

        Q2TP = [wp.tile([P, Q], BF16, tag=f"Q2T{p}", name=f"Q2T{p}") for p in range(4)]
        K2TP = [wp.tile([P, TCTX], BF16, tag=f"K2T{p}", name=f"K2T{p}") for p in range(4)]
        VP2 = wp.tile([TCTX, H, D + 1], BF16, tag="VP2")
        nc.vector.memset(VP2[:, :, D:D + 1], 1.0)

        for p4 in range(4):
            for qb in range(2):
                ps = pp.tile([P, 512], F32, tag="ps2")
                for c in range(CC):
                    nc.tensor.matmul(ps[:], lhsT=WQ2[c][:, p4 * P:(p4 + 1) * P],
                                     rhs=H1NT[:, c, qb * 512:(qb + 1) * 512],
                                     start=(c == 0), stop=(c == CC - 1))
                nc.vector.tensor_copy(out=Q2TP[p4][:, qb * 512:(qb + 1) * 512], in_=ps[:])
            psk = pp.tile([P, TCTX], F32, tag="psk", bufs=1)
            for c in range(CC):
                nc.tensor.matmul(psk[:], lhsT=WK2[c][:, p4 * P:(p4 + 1) * P], rhs=CTX[c][:],
                                 start=(c == 0), stop=(c == CC - 1))
            nc.vector.tensor_copy(out=K2TP[p4][:], in_=psk[:])
        psv = pp.tile([TCTX, 512], F32, tag="psv", bufs=1)
        for c in range(CC):
            nc.tensor.matmul(psv[:], lhsT=CTX[c][:], rhs=WV2[c][:],
                             start=(c == 0), stop=(c == CC - 1))
        nc.vector.tensor_copy(out=VP2[:, :, 0:D],
                              in_=psv[:].rearrange("p (h d) -> p h d", h=H))

        for qb in range(2):
            for h in range(H):
                p4, hh = h // 2, h % 2
                s2 = pp.tile([TCTX, 512], F32, tag="s2")
                nc.tensor.matmul(
                    s2[:],
                    lhsT=K2TP[p4][hh * D:(hh + 1) * D, :],
                    rhs=Q2TP[p4][hh * D:(hh + 1) * D, qb * 512:(qb + 1) * 512],
                    start=True, stop=True)
                p2 = sb.tile([TCTX, 512], BF16, tag="p2")
                nc.scalar.activation(out=p2[:], in_=s2[:], func=AF.Exp, scale=SCALE)
                o65 = pp.tile([D + 1, 512], F32, tag="o65x")
                nc.tensor.matmul(o65[:], lhsT=VP2[:, h, :], rhs=p2[:], start=True, stop=True)
                den = sb.tile([1, 512], F32, tag="den2")
                nc.vector.tensor_copy(out=den[:], in_=o65[D:D + 1, :])
                dbc = sb.tile([D, 512], F32, tag="dbc2")
                nc.gpsimd.partition_broadcast(out_ap=dbc[:], in_ap=den[:], channels=D)
                rc = sb.tile([D, 512], F32, tag="rc2")
                nc.vector.reciprocal_approx_fast(out=rc[:], in_=dbc[:])
                nc.vector.tensor_mul(
                    out=O2T[hh * D:(hh + 1) * D, p4, qb * 512:(qb + 1) * 512],
                    in0=o65[0:D, :], in1=rc[:])

    with tc.tile_pool(name="wo2p", bufs=1) as wp, \
         tc.tile_pool(name="pr2ps", bufs=3, space="PSUM") as pp:
        WO2 = load_w(wp, "wo2")
        for qb in range(2):
            for e in range(CC):
                ps = pp.tile([P, 512], F32, tag="ps")
                for c in range(CC):
                    nc.tensor.matmul(ps[:], lhsT=WO2[c][:, e * P:(e + 1) * P],
                                     rhs=O2T[:, c, qb * 512:(qb + 1) * 512],
                                     start=(c == 0), stop=(c == CC - 1))
                nc.vector.tensor_add(out=H1T[:, e, qb * 512:(qb + 1) * 512],
                                     in0=ps[:], in1=H1T[:, e, qb * 512:(qb + 1) * 512])
    H2T = H1T  # h2 written in place; H1T now holds the post-cross-attn residual

    cm_mid.__exit__(None, None, None)
    cm_ffn, pool_ffn = open_pool("pool_ffn", side="right")  # H2NT, FF : to the end
    H2NT = pool_ffn.tile([P, CC, Q], BF16, tag="H2NT")
    with tc.tile_pool(name="ln3", bufs=1) as lp:
        layer_norm_T(H2T, H2NT, lp)

    # ---------------- Phase G: GEGLU FFN + out proj + residual ----------------
    FB = 16  # 2048/128 blocks in each geglu half
    FF = pool_ffn.tile([P, FB, Q], BF16, tag="FF")
    with tc.tile_pool(name="gwp", bufs=1) as wp, \
         tc.tile_pool(name="ffps", bufs=4, space="PSUM") as pp, \
         tc.tile_pool(name="ffsb", bufs=4) as sb:
        GW = load_w(wp, "gw", cols=8 * DIM)
        for qb in range(2):
            for fb in range(FB):
                psy = pp.tile([P, 512], F32, tag="psy")
                psg = pp.tile([P, 512], F32, tag="psg")
                for c in range(CC):
                    nc.tensor.matmul(psy[:], lhsT=GW[c][:, fb * P:(fb + 1) * P],
                                     rhs=H2NT[:, c, qb * 512:(qb + 1) * 512],
                                     start=(c == 0), stop=(c == CC - 1))
                for c in range(CC):
                    nc.tensor.matmul(psg[:], lhsT=GW[c][:, 4 * DIM + fb * P:4 * DIM + (fb + 1) * P],
                                     rhs=H2NT[:, c, qb * 512:(qb + 1) * 512],
                                     start=(c == 0), stop=(c == CC - 1))
                ga = sb.tile([P, 512], BF16, tag="ga")
                nc.scalar.activation(out=ga[:], in_=psg[:], func=AF.Gelu_apprx_tanh)
                nc.vector.tensor_mul(out=FF[:, fb, qb * 512:(qb + 1) * 512],
                                     in0=psy[:], in1=ga[:])

    with tc.tile_pool(name="owp", bufs=1) as wp, \
         tc.tile_pool(name="outps", bufs=3, space="PSUM") as pp, \
         tc.tile_pool(name="outsb", bufs=3) as sb:
        OW = load_w(wp, "ow", rows=4 * DIM)
        for qb in range(2):
            for e in range(CC):
                ps = pp.tile([P, 512], F32, tag="ps")
                for f in range(FB):
                    nc.tensor.matmul(ps[:], lhsT=OW[f][:, e * P:(e + 1) * P],
                                     rhs=FF[:, f, qb * 512:(qb + 1) * 512],
                                     start=(f == 0), stop=(f == FB - 1))
                fin = sb.tile([P, 512], F32, tag="fin")
                nc.vector.tensor_add(out=fin[:], in0=ps[:],
                                     in1=H2T[:, e, qb * 512:(qb + 1) * 512])
                nc.sync.dma_start(out=a["outT"][e * P:(e + 1) * P, qb * 512:(qb + 1) * 512],
                                  in_=fin[:])

    cm_ffn.__exit__(None, None, None)
    cm_h1.__exit__(None, None, None)


def build_program():
    nc = bacc.Bacc("TRN2", target_bir_lowering=False, debug=False)
    a = {}

    def din(name, shape, dt):
        a[name] = nc.dram_tensor(name, list(shape), dt, kind="ExternalInput").ap()

    din("xT", [DIM, S], BF16)
    din("xnat", [S, DIM], BF16)
    din("xresT", [DIM, Q], F32)
    din("ctxT", [DIM, TCTX], BF16)
    for w in ["wq1", "wk1", "wv1", "wo1", "wq2", "wk2", "wv2", "wo2"]:
        din(w, [DIM, DIM], BF16)
    din("gw", [DIM, 8 * DIM], BF16)
    din("ow", [4 * DIM, DIM], BF16)
    a["outT"] = nc.dram_tensor("outT", [DIM, Q], F32, kind="ExternalOutput").ap()

    with tile.TileContext(nc) as tc:
        with ExitStack() as ctx:
            _body(ctx, tc, a)
    nc.compile()
    return nc


def host_prepare(inputs):
    """Fold LN affine params into weights, cast, slice/permute per core."""
    f = lambda t: np.asarray(t, dtype=np.float32)
    x = f(inputs["x"])
    context = f(inputs["context"])
    g1 = f(inputs["ln1_g"])[:, None]
    g2 = f(inputs["ln2_g"])[:, None]
    g3 = f(inputs["ln3_g"])[:, None]
    for nm in ["ln1_b", "ln2_b", "ln3_b", "bo1", "bo2", "geglu_b", "out_b"]:
        assert not np.any(f(inputs[nm])), f"nonzero bias {nm} not supported"

    bf = ml_dtypes.bfloat16
    weights = {
        "wq1": (g1 * f(inputs["wq1"])).astype(bf),
        "wk1": (g1 * f(inputs["wk1"])).astype(bf),
        "wv1": (g1 * f(inputs["wv1"])).astype(bf),
        "wo1": f(inputs["wo1"]).astype(bf),
        "wq2": (g2 * f(inputs["wq2"])).astype(bf),
        "wk2": f(inputs["wk2"]).astype(bf),
        "wv2": f(inputs["wv2"]).astype(bf),
        "wo2": f(inputs["wo2"]).astype(bf),
        "gw": (g3 * f(inputs["geglu_w"])).astype(bf),
        "ow": f(inputs["out_w"]).astype(bf),
    }

    in_maps = []
    for core in range(NCORES):
        b = core // 4
        q0 = (core % 4) * Q
        perm = np.concatenate([np.arange(q0, q0 + Q), np.delete(np.arange(S), np.s_[q0:q0 + Q])])
        xc = x[b][perm]                       # [S, DIM], own queries first
        m = dict(weights)
        m["xT"] = np.ascontiguousarray(xc.T).astype(bf)
        # bn_stats tile permutation: row tb*128+p holds token p*32+tb
        m["xnat"] = np.ascontiguousarray(
            xc.reshape(P, 32, DIM).transpose(1, 0, 2).reshape(S, DIM)).astype(bf)
        m["xresT"] = np.ascontiguousarray(x[b, q0:q0 + Q].T)
        m["ctxT"] = np.ascontiguousarray(context[b].T).astype(bf)
        in_maps.append(m)
    return in_maps


def kernel(**inputs):
    if "nc" not in _CACHE:
        _CACHE["nc"] = build_program()
    nc = _CACHE["nc"]
    in_maps = host_prepare(inputs)
    res = run_bass_kernel_spmd(nc, in_maps, list(range(NCORES)))
    out = np.zeros((B, S, DIM), dtype=np.float32)
    for core in range(NCORES):
        b = core // 4
        q0 = (core % 4) * Q
        out[b, q0:q0 + Q, :] = res.results[core]["outT"].T
    return out


# revision 15
# speedup vs baseline: 26.6407x; 1.0320x over previous
"""Trainium2 Bass kernel for a BasicTransformerBlock (self-attn + cross-attn + GEGLU FFN).

Sharding: pure data-parallel over (batch, query-rows). 8 cores = 2 batches x 4
query-slices of 1024 rows. Only the self-attention K/V path needs all 4096
tokens of a batch element, and K/V are recomputed per core from the (shared)
input x, so there are no collectives at all.

On-device dataflow is kept in "transposed" (feature-on-partition) layout
throughout, which makes every bias/scale a per-partition op and makes the
attention matmuls natural:
  xnT [c,t]  -> qT/kT [d,t] (per head-pair tiles) , v natural [t,d]
  S^T [k,q] = kT^T@qT ; P = exp(S^T/8) ; o65 = [v|1]^T @ P  (denominator rides
  as output row 64) ; normalize with broadcast reciprocal; project, residual,
  LN (stats via gpsimd partition_all_reduce), GEGLU, out-proj.
"""

import numpy as np
import ml_dtypes
from contextlib import ExitStack

import concourse.bass as bass
import concourse.bass_isa as bass_isa
import concourse.tile as tile
from concourse import bacc, mybir
from concourse.bass_utils import run_bass_kernel_spmd

AF = mybir.ActivationFunctionType
BF16 = mybir.dt.bfloat16
F32 = mybir.dt.float32

DIM = 512
H = 8
D = 64
B = 2
S = 4096
TCTX = 77
NCORES = 8
Q = 1024          # query rows per core
P = 128
CC = DIM // P     # contraction chunks of 128
EPS = 1e-5
SCALE = D ** -0.5

_CACHE = {}


def _bcast_dram_ap(ap, nparts):
    """DMA source AP that broadcasts a DRAM row across nparts partitions."""
    return bass.AP(tensor=ap.tensor, offset=ap.offset, ap=[[0, nparts]] + list(ap.ap))


def _body(ctx, tc, a):
    nc = tc.nc
    persist = ctx.enter_context(tc.tile_pool(name="persist", bufs=1))

    def open_pool(name, side="left"):
        cm = tc.tile_pool(name=name, bufs=1, side=side)
        pool = cm.__enter__()
        return cm, pool

    cm_x, pool_x = open_pool("pool_x")          # m_b, r_b, XT : dies after QKV
    eps_t = persist.tile([P, 1], F32, tag="eps")
    nc.vector.memset(eps_t[:], EPS)

    # ---------------- Phase A: LN1 stats (bn_stats over permuted x-natural) ---
    m_b = pool_x.tile([P, S], BF16, tag="m_b")
    r_b = pool_x.tile([P, S], BF16, tag="r_b")
    with tc.tile_pool(name="lnA", bufs=4) as lp:
        mv = lp.tile([P, 32, 2], F32, tag="mv", bufs=1)
        for tb in range(32):
            xt = lp.tile([P, DIM], BF16, tag="xnt")
            nc.sync.dma_start(out=xt, in_=a["xnat"][tb * P:(tb + 1) * P, :])
            st = lp.tile([P, 6], F32, tag="st6")
            nc.vector.bn_stats(out=st, in_=xt)
            nc.vector.bn_aggr(out=mv[:, tb, :], in_=st)
        lnv = lp.tile([P, 32], F32, tag="lnv", bufs=1)
        nc.scalar.activation(out=lnv, in_=mv[:, :, 1], func=AF.Ln, bias=eps_t[:])
        rst = lp.tile([P, 32], BF16, tag="rst", bufs=1)
        nc.scalar.activation(out=rst, in_=lnv, func=AF.Exp, scale=-0.5)
        mbf = lp.tile([P, 32], BF16, tag="mbf", bufs=1)
        nc.vector.tensor_copy(out=mbf, in_=mv[:, :, 0])
        # [128,32] (p-major == t-order thanks to host permute) -> [1,4096] rows
        m_row = lp.tile([1, S], BF16, tag="m_row", bufs=1)
        r_row = lp.tile([1, S], BF16, tag="r_row", bufs=1)
        nc.sync.dma_start(out=m_row, in_=mbf[:])
        nc.sync.dma_start(out=r_row, in_=rst[:])
        nc.gpsimd.partition_broadcast(out_ap=m_b[:], in_ap=m_row[:], channels=P)
        nc.gpsimd.partition_broadcast(out_ap=r_b[:], in_ap=r_row[:], channels=P)

    # ---------------- Phase B: load xT, normalize in place -> xnT -------------
    XT = []
    for c in range(CC):
        t = pool_x.tile([P, S], BF16, tag=f"XT{c}")
        nc.sync.dma_start(out=t, in_=a["xT"][c * P:(c + 1) * P, :])
        XT.append(t)
    for c in range(CC):
        nc.vector.tensor_sub(out=XT[c][:], in0=XT[c][:], in1=m_b[:])
        nc.vector.tensor_mul(out=XT[c][:], in0=XT[c][:], in1=r_b[:])

    # ---------------- Phase C: QKV projections --------------------------------
    cm_at, pool_at = open_pool("pool_at", side="right")  # KT, QT, VP, O1T
    KTP = [pool_at.tile([P, S], BF16, tag=f"KT{p}", name=f"KT{p}") for p in range(4)]
    QTP = [pool_at.tile([P, Q], BF16, tag=f"QT{p}", name=f"QT{p}") for p in range(4)]
    VP = pool_at.tile([P, 32, H, D + 1], BF16, tag="VP")
    nc.vector.memset(VP[:, :, :, D:D + 1], 1.0)

    def load_w(pool, name, rows=DIM, cols=DIM):
        ts = []
        for c in range(rows // P):
            t = pool.tile([P, cols], BF16, tag=f"{name}{c}")
            nc.sync.dma_start(out=t, in_=a[name][c * P:(c + 1) * P, :])
            ts.append(t)
        return ts

    with tc.tile_pool(name="w1", bufs=1) as wp, \
         tc.tile_pool(name="qkvps", bufs=4, space="PSUM") as pp:
        WQ = load_w(wp, "wq1")
        WK = load_w(wp, "wk1")
        WV = load_w(wp, "wv1")
        for p4 in range(4):
            for qb in range(2):
                ps = pp.tile([P, 512], F32, tag="ps")
                for c in range(CC):
                    nc.tensor.matmul(ps[:], lhsT=WQ[c][:, p4 * P:(p4 + 1) * P],
                                     rhs=XT[c][:, qb * 512:(qb + 1) * 512],
                                     start=(c == 0), stop=(c == CC - 1))
                nc.vector.tensor_copy(out=QTP[p4][:, qb * 512:(qb + 1) * 512], in_=ps[:])
            for kb8 in range(8):
                ps = pp.tile([P, 512], F32, tag="ps")
                for c in range(CC):
                    nc.tensor.matmul(ps[:], lhsT=WK[c][:, p4 * P:(p4 + 1) * P],
                                     rhs=XT[c][:, kb8 * 512:(kb8 + 1) * 512],
                                     start=(c == 0), stop=(c == CC - 1))
                if kb8 % 2 == 0:
                    nc.vector.tensor_copy(out=KTP[p4][:, kb8 * 512:(kb8 + 1) * 512], in_=ps[:])
                else:
                    nc.scalar.copy(out=KTP[p4][:, kb8 * 512:(kb8 + 1) * 512], in_=ps[:])
        for tb in range(32):
            ps = pp.tile([P, 512], F32, tag="ps")
            for c in range(CC):
                nc.tensor.matmul(ps[:], lhsT=XT[c][:, tb * P:(tb + 1) * P], rhs=WV[c][:],
                                 start=(c == 0), stop=(c == CC - 1))
            nc.vector.tensor_copy(out=VP[:, tb, :, 0:D],
                                  in_=ps[:].rearrange("p (h d) -> p h d", h=H))

    # ---------------- Phase D: self-attention ---------------------------------
    cm_x.__exit__(None, None, None)   # xnT / m_b / r_b no longer needed
    O1T = pool_at.tile([P, CC, Q], BF16, tag="O1T")
    with tc.tile_pool(name="spool", bufs=2, space="PSUM") as spool, \
         tc.tile_pool(name="opool", bufs=4, space="PSUM") as opool, \
         tc.tile_pool(name="ppool", bufs=4) as ppool, \
         tc.tile_pool(name="npool", bufs=4) as npool:
        for qb in range(2):
            for p4 in range(4):
                oo = []
                for hh in range(2):
                    o65 = opool.tile([D + 1, 512], F32, tag="o65", name=f"o65_{qb}_{p4}_{hh}")
                    oo.append(o65)
                pend = None  # software pipeline: attnV(kb) issues after scores(kb+1)
                for kb in range(32):
                    # both heads' scores into one 2-bank psum region, one exp
                    s2t = spool.tile([P, 2, 512], F32, tag="S")
                    for hh in range(2):
                        nc.tensor.matmul(
                            s2t[:, hh, :],
                            lhsT=KTP[p4][hh * D:(hh + 1) * D, kb * P:(kb + 1) * P],
                            rhs=QTP[p4][hh * D:(hh + 1) * D, qb * 512:(qb + 1) * 512],
                            start=True, stop=True)
                    pA = ppool.tile([P, 2, 512], BF16, tag="P")
                    nc.scalar.activation(out=pA[:], in_=s2t[:], func=AF.Exp, scale=SCALE)
                    if pend is not None:
                        pkb, ppA = pend
                        for hh in range(2):
                            nc.tensor.matmul(oo[hh][:], lhsT=VP[:, pkb, 2 * p4 + hh, :],
                                             rhs=ppA[:, hh, :],
                                             start=(pkb == 0), stop=False)
                    pend = (kb, pA)
                pkb, ppA = pend
                for hh in range(2):
                    nc.tensor.matmul(oo[hh][:], lhsT=VP[:, pkb, 2 * p4 + hh, :],
                                     rhs=ppA[:, hh, :], start=False, stop=True)
                for hh in range(2):
                    den = npool.tile([1, 512], F32, tag="den")
                    nc.vector.tensor_copy(out=den[:], in_=oo[hh][D:D + 1, :])
                    dbc = npool.tile([D, 512], F32, tag="dbc")
                    nc.gpsimd.partition_broadcast(out_ap=dbc[:], in_ap=den[:], channels=D)
                    rc = npool.tile([D, 512], F32, tag="rc")
                    nc.vector.reciprocal_approx_fast(out=rc[:], in_=dbc[:])
                    nc.vector.tensor_mul(
                        out=O1T[hh * D:(hh + 1) * D, p4, qb * 512:(qb + 1) * 512],
                        in0=oo[hh][0:D, :], in1=rc[:])

    # ---------------- Phase E: out-proj 1 + residual -> h1T (f32) -------------
    cm_h1, pool_h1 = open_pool("pool_h1")       # XRES + running residual HT (lives to end)
    XRES = []
    for e in range(CC):
        t = pool_h1.tile([P, Q], F32, tag=f"XRES{e}")
        nc.sync.dma_start(out=t, in_=a["xresT"][e * P:(e + 1) * P, :])
        XRES.append(t)

    H1T = pool_h1.tile([P, CC, Q], F32, tag="H1T")
    with tc.tile_pool(name="wo1p", bufs=1) as wp, \
         tc.tile_pool(name="prps", bufs=3, space="PSUM") as pp:
        WO1 = load_w(wp, "wo1")
        for qb in range(2):
            for e in range(CC):
                ps = pp.tile([P, 512], F32, tag="ps")
                for c in range(CC):
                    nc.tensor.matmul(ps[:], lhsT=WO1[c][:, e * P:(e + 1) * P],
                                     rhs=O1T[:, c, qb * 512:(qb + 1) * 512],
                                     start=(c == 0), stop=(c == CC - 1))
                nc.vector.tensor_add(out=H1T[:, e, qb * 512:(qb + 1) * 512],
                                     in0=ps[:], in1=XRES[e][:, qb * 512:(qb + 1) * 512])

    # ---------------- layer norm in transposed layout (stats over partitions) -
    ones_b = persist.tile([P, 1], BF16, tag="ones_b")
    nc.vector.memset(ones_b[:], 1.0)

    def layer_norm_T(HT, OUT_BF, lp):
        # per-token sums over the feature axis via ones-stationary matmuls
        # (partition reduce on PE); stats stay token-on-free so the broadcast
        # is a plain partition_broadcast.
        with tc.tile_pool(name="lnps", bufs=1, space="PSUM") as pp:
            ps1 = pp.tile([1, Q], F32, tag="lnps1")
            ps2 = pp.tile([1, Q], F32, tag="lnps2")
            for c in range(CC):
                hbf = lp.tile([P, Q], BF16, tag="hbf")
                nc.vector.tensor_copy(out=hbf[:], in_=HT[:, c, :])
                sq = lp.tile([P, Q], BF16, tag="sq")
                nc.vector.tensor_mul(out=sq[:], in0=hbf[:], in1=hbf[:])
                for qh in range(2):
                    nc.tensor.matmul(ps1[:, qh * 512:(qh + 1) * 512], lhsT=ones_b[:],
                                     rhs=hbf[:, qh * 512:(qh + 1) * 512],
                                     start=(c == 0), stop=(c == CC - 1))
                    nc.tensor.matmul(ps2[:, qh * 512:(qh + 1) * 512], lhsT=ones_b[:],
                                     rhs=sq[:, qh * 512:(qh + 1) * 512],
                                     start=(c == 0), stop=(c == CC - 1))
            m_row = lp.tile([1, Q], F32, tag="m_row2")
            nc.vector.tensor_scalar(out=m_row[:], in0=ps1[:], scalar1=1.0 / DIM,
                                    scalar2=None, op0=mybir.AluOpType.mult)
            v1 = lp.tile([1, Q], F32, tag="v1r")
            nc.vector.tensor_scalar(out=v1[:], in0=ps2[:], scalar1=1.0 / DIM,
                                    scalar2=None, op0=mybir.AluOpType.mult)
            m2 = lp.tile([1, Q], F32, tag="m2r")
            nc.vector.tensor_mul(out=m2[:], in0=m_row[:], in1=m_row[:])
            var = lp.tile([1, Q], F32, tag="varr")
            nc.vector.tensor_sub(out=var[:], in0=v1[:], in1=m2[:])
            lnv = lp.tile([1, Q], F32, tag="lnvr")
            nc.scalar.activation(out=lnv[:], in_=var[:], func=AF.Ln, bias=eps_t[0:1, :])
            r_row = lp.tile([1, Q], F32, tag="r_row2")
            nc.scalar.activation(out=r_row[:], in_=lnv[:], func=AF.Exp, scale=-0.5)
        mb = lp.tile([P, Q], F32, tag="mb2")
        rb = lp.tile([P, Q], F32, tag="rb2")
        nc.gpsimd.partition_broadcast(out_ap=mb[:], in_ap=m_row[:], channels=P)
        nc.gpsimd.partition_broadcast(out_ap=rb[:], in_ap=r_row[:], channels=P)
        for e in range(CC):
            tmp = lp.tile([P, Q], F32, tag="tmp")
            nc.vector.tensor_sub(out=tmp[:], in0=HT[:, e, :], in1=mb[:])
            nc.vector.tensor_mul(out=OUT_BF[:, e, :], in0=tmp[:], in1=rb[:])

    cm_at.__exit__(None, None, None)  # attention operands done
    cm_mid, pool_mid = open_pool("pool_mid", side="right")  # H1NT, O2T : dies after proj2
    H1NT = pool_mid.tile([P, CC, Q], BF16, tag="H1NT")
    with tc.tile_pool(name="ln2", bufs=1) as lp:
        layer_norm_T(H1T, H1NT, lp)

    # ---------------- Phase F: cross-attention --------------------------------
    O2T = pool_mid.tile([P, CC, Q], BF16, tag="O2T")
    with tc.tile_pool(name="w2", bufs=1) as wp, \
         tc.tile_pool(name="c2ps", bufs=2, space="PSUM") as pp, \
         tc.tile_pool(name="c2sb", bufs=4) as sb:
        WQ2 = load_w(wp, "wq2")
        WK2 = load_w(wp, "wk2")
        WV2 = load_w(wp, "wv2")
        CTX = []
        for c in range(CC):
            t = wp.tile([P, TCTX], BF16, tag=f"CTX{c}")
            nc.sync.dma_start(out=t, in_=a["ctxT"][c * P:(c + 1) * P, :])
            CTX.append(t)

        Q2TP = [wp.tile([P, Q], BF16, tag=f"Q2T{p}", name=f"Q2T{p}") for p in range(4)]
        K2TP = [wp.tile([P, TCTX], BF16, tag=f"K2T{p}", name=f"K2T{p}") for p in range(4)]
        VP2 = wp.tile([TCTX, H, D + 1], BF16, tag="VP2")
        nc.vector.memset(VP2[:, :, D:D + 1], 1.0)

        for p4 in range(4):
            for qb in range(2):
                ps = pp.tile([P, 512], F32, tag="ps2")
                for c in range(CC):
                    nc.tensor.matmul(ps[:], lhsT=WQ2[c][:, p4 * P:(p4 + 1) * P],
                                     rhs=H1NT[:, c, qb * 512:(qb + 1) * 512],
                                     start=(c == 0), stop=(c == CC - 1))
                nc.vector.tensor_copy(out=Q2TP[p4][:, qb * 512:(qb + 1) * 512], in_=ps[:])
            psk = pp.tile([P, TCTX], F32, tag="psk", bufs=1)
            for c in range(CC):
                nc.tensor.matmul(psk[:], lhsT=WK2[c][:, p4 * P:(p4 + 1) * P], rhs=CTX[c][:],
                                 start=(c == 0), stop=(c == CC - 1))
            nc.vector.tensor_copy(out=K2TP[p4][:], in_=psk[:])
        psv = pp.tile([TCTX, 512], F32, tag="psv", bufs=1)
        for c in range(CC):
            nc.tensor.matmul(psv[:], lhsT=CTX[c][:], rhs=WV2[c][:],
                             start=(c == 0), stop=(c == CC - 1))
        nc.vector.tensor_copy(out=VP2[:, :, 0:D],
                              in_=psv[:].rearrange("p (h d) -> p h d", h=H))

        for qb in range(2):
            for h in range(H):
                p4, hh = h // 2, h % 2
                s2 = pp.tile([TCTX, 512], F32, tag="s2")
                nc.tensor.matmul(
                    s2[:],
                    lhsT=K2TP[p4][hh * D:(hh + 1) * D, :],
                    rhs=Q2TP[p4][hh * D:(hh + 1) * D, qb * 512:(qb + 1) * 512],
                    start=True, stop=True)
                p2 = sb.tile([TCTX, 512], BF16, tag="p2")
                nc.scalar.activation(out=p2[:], in_=s2[:], func=AF.Exp, scale=SCALE)
                o65 = pp.tile([D + 1, 512], F32, tag="o65x")
                nc.tensor.matmul(o65[:], lhsT=VP2[:, h, :], rhs=p2[:], start=True, stop=True)
                den = sb.tile([1, 512], F32, tag="den2")
                nc.vector.tensor_copy(out=den[:], in_=o65[D:D + 1, :])
                dbc = sb.tile([D, 512], F32, tag="dbc2")
                nc.gpsimd.partition_broadcast(out_ap=dbc[:], in_ap=den[:], channels=D)
                rc = sb.tile([D, 512], F32, tag="rc2")
                nc.vector.reciprocal_approx_fast(out=rc[:], in_=dbc[:])
                nc.vector.tensor_mul(
                    out=O2T[hh * D:(hh + 1) * D, p4, qb * 512:(qb + 1) * 512],
                    in0=o65[0:D, :], in1=rc[:])

    with tc.tile_pool(name="wo2p", bufs=1) as wp, \
         tc.tile_pool(name="pr2ps", bufs=3, space="PSUM") as pp:
        WO2 = load_w(wp, "wo2")
        for qb in range(2):
            for e in range(CC):
                ps = pp.tile([P, 512], F32, tag="ps")
                for c in range(CC):
                    nc.tensor.matmul(ps[:], lhsT=WO2[c][:, e * P:(e + 1) * P],
                                     rhs=O2T[:, c, qb * 512:(qb + 1) * 512],
                                     start=(c == 0), stop=(c == CC - 1))
                nc.vector.tensor_add(out=H1T[:, e, qb * 512:(qb + 1) * 512],
                                     in0=ps[:], in1=H1T[:, e, qb * 512:(qb + 1) * 512])
    H2T = H1T  # h2 written in place; H1T now holds the post-cross-attn residual

    cm_mid.__exit__(None, None, None)
    cm_ffn, pool_ffn = open_pool("pool_ffn", side="right")  # H2NT, FF : to the end
    H2NT = pool_ffn.tile([P, CC, Q], BF16, tag="H2NT")
    with tc.tile_pool(name="ln3", bufs=1) as lp:
        layer_norm_T(H2T, H2NT, lp)

    # ---------------- Phase G: GEGLU FFN + out proj + residual ----------------
    FB = 16  # 2048/128 blocks in each geglu half
    FF = pool_ffn.tile([P, FB, Q], BF16, tag="FF")
    with tc.tile_pool(name="gwp", bufs=1) as wp, \
         tc.tile_pool(name="ffps", bufs=4, space="PSUM") as pp, \
         tc.tile_pool(name="ffsb", bufs=4) as sb:
        GW = load_w(wp, "gw", cols=8 * DIM)
        for qb in range(2):
            for fb in range(FB):
                psy = pp.tile([P, 512], F32, tag="psy")
                psg = pp.tile([P, 512], F32, tag="psg")
                for c in range(CC):
                    nc.tensor.matmul(psy[:], lhsT=GW[c][:, fb * P:(fb + 1) * P],
                                     rhs=H2NT[:, c, qb * 512:(qb + 1) * 512],
                                     start=(c == 0), stop=(c == CC - 1))
                for c in range(CC):
                    nc.tensor.matmul(psg[:], lhsT=GW[c][:, 4 * DIM + fb * P:4 * DIM + (fb + 1) * P],
                                     rhs=H2NT[:, c, qb * 512:(qb + 1) * 512],
                                     start=(c == 0), stop=(c == CC - 1))
                ga = sb.tile([P, 512], BF16, tag="ga")
                nc.scalar.activation(out=ga[:], in_=psg[:], func=AF.Gelu_apprx_tanh)
                nc.vector.tensor_mul(out=FF[:, fb, qb * 512:(qb + 1) * 512],
                                     in0=psy[:], in1=ga[:])

    with tc.tile_pool(name="owp", bufs=1) as wp, \
         tc.tile_pool(name="outps", bufs=3, space="PSUM") as pp, \
         tc.tile_pool(name="outsb", bufs=3) as sb:
        OW = load_w(wp, "ow", rows=4 * DIM)
        for qb in range(2):
            for e in range(CC):
                ps = pp.tile([P, 512], F32, tag="ps")
                for f in range(FB):
                    nc.tensor.matmul(ps[:], lhsT=OW[f][:, e * P:(e + 1) * P],
                                     rhs=FF[:, f, qb * 512:(qb + 1) * 512],
                                     start=(f == 0), stop=(f == FB - 1))
                fin = sb.tile([P, 512], F32, tag="fin")
                nc.vector.tensor_add(out=fin[:], in0=ps[:],
                                     in1=H2T[:, e, qb * 512:(qb + 1) * 512])
                nc.sync.dma_start(out=a["outT"][e * P:(e + 1) * P, qb * 512:(qb + 1) * 512],
                                  in_=fin[:])

    cm_ffn.__exit__(None, None, None)
    cm_h1.__exit__(None, None, None)


def build_program():
    nc = bacc.Bacc("TRN2", target_bir_lowering=False, debug=False)
    a = {}

    def din(name, shape, dt):
        a[name] = nc.dram_tensor(name, list(shape), dt, kind="ExternalInput").ap()

    din("xT", [DIM, S], BF16)
    din("xnat", [S, DIM], BF16)
    din("xresT", [DIM, Q], F32)
    din("ctxT", [DIM, TCTX], BF16)
    for w in ["wq1", "wk1", "wv1", "wo1", "wq2", "wk2", "wv2", "wo2"]:
        din(w, [DIM, DIM], BF16)
    din("gw", [DIM, 8 * DIM], BF16)
    din("ow", [4 * DIM, DIM], BF16)
    a["outT"] = nc.dram_tensor("outT", [DIM, Q], F32, kind="ExternalOutput").ap()

    with tile.TileContext(nc) as tc:
        with ExitStack() as ctx:
            _body(ctx, tc, a)
    nc.compile()
    return nc


def host_prepare(inputs):
    """Fold LN affine params into weights, cast, slice/permute per core."""
    f = lambda t: np.asarray(t, dtype=np.float32)
    x = f(inputs["x"])
    context = f(inputs["context"])
    g1 = f(inputs["ln1_g"])[:, None]
    g2 = f(inputs["ln2_g"])[:, None]
    g3 = f(inputs["ln3_g"])[:, None]
    for nm in ["ln1_b", "ln2_b", "ln3_b", "bo1", "bo2", "geglu_b", "out_b"]:
        assert not np.any(f(inputs[nm])), f"nonzero bias {nm} not supported"

    bf = ml_dtypes.bfloat16
    weights = {
        "wq1": (g1 * f(inputs["wq1"])).astype(bf),
        "wk1": (g1 * f(inputs["wk1"])).astype(bf),
        "wv1": (g1 * f(inputs["wv1"])).astype(bf),
        "wo1": f(inputs["wo1"]).astype(bf),
        "wq2": (g2 * f(inputs["wq2"])).astype(bf),
        "wk2": f(inputs["wk2"]).astype(bf),
        "wv2": f(inputs["wv2"]).astype(bf),
        "wo2": f(inputs["wo2"]).astype(bf),
        "gw": (g3 * f(inputs["geglu_w"])).astype(bf),
        "ow": f(inputs["out_w"]).astype(bf),
    }

    in_maps = []
    for core in range(NCORES):
        b = core // 4
        q0 = (core % 4) * Q
        perm = np.concatenate([np.arange(q0, q0 + Q), np.delete(np.arange(S), np.s_[q0:q0 + Q])])
        xc = x[b][perm]                       # [S, DIM], own queries first
        m = dict(weights)
        m["xT"] = np.ascontiguousarray(xc.T).astype(bf)
        # bn_stats tile permutation: row tb*128+p holds token p*32+tb
        m["xnat"] = np.ascontiguousarray(
            xc.reshape(P, 32, DIM).transpose(1, 0, 2).reshape(S, DIM)).astype(bf)
        m["xresT"] = np.ascontiguousarray(x[b, q0:q0 + Q].T)
        m["ctxT"] = np.ascontiguousarray(context[b].T).astype(bf)
        in_maps.append(m)
    return in_maps


def kernel(**inputs):
    if "nc" not in _CACHE:
        _CACHE["nc"] = build_program()
    nc = _CACHE["nc"]
    in_maps = host_prepare(inputs)
    res = run_bass_kernel_spmd(nc, in_maps, list(range(NCORES)))
    out = np.zeros((B, S, DIM), dtype=np.float32)
    for core in range(NCORES):
        b = core // 4
        q0 = (core % 4) * Q
        out[b, q0:q0 + Q, :] = res.results[core]["outT"].T
    return out
